# revision 20
# baseline (speedup 1.0000x reference)
"""GNN message-passing kernel for 8 TRN2 NeuronCores (Bass/Tile, SPMD).

Takes the FULL inputs of nn_Base_40793599378196 and returns the FULL
[512, 130] output. Internally:

- Nodes/graphs sharded by graph: core c owns nodes [c*8192, (c+1)*8192).
  Weights replicated. Per layer y = h @ wn is computed locally (node-major
  bf16), AllGathered into a full [65536, 128] DRAM table.
- Aggregation avoids both the random-256B-HBM-read drain floor (~105ns per
  descriptor per SDMA engine) and the one-hot/scatter-matmul volume:
  * Each 32768-node half of the table is DMA'd contiguously into an SBUF
    table (8MB, token t = y row (t%128)*256 + t//128 at partition t%128,
    stripe t//128 -- so the idx for row r is (r%256)*128 + r//256).
  * Edges are organized per (dst node, src half) into K=4 fixed slots;
    slot-slab streams are gathered EDGE-MAJOR with a non-transpose
    SBUF-source dma_gather (constructed directly -- the bass helper only
    allows SBUF sources with transpose=True, but the firmware supports
    non-transpose too, avoiding the xbar so the 4 SWDGE queues can
    generate descriptors concurrently; SBUF reads make the drain cheap).
  * The K slabs accumulate into agg[node%128, node//128, feat] with plain
    DVE adds; per 512-node group a transpose-matmul (agg_chunk^T @ I)
    accumulates agg into the root-transform PSUM group. Pad slots gather
    token 0 and are cancelled by a rank-1 correction matmul.
  * Overflow edges (degree > K per half, ~2.5%) are gathered from DRAM
    y_full (edge-major chunks per 256-node spill group) and scattered via
    one-hot (iota256 is_equal) matmuls into the same PSUM groups.
- Gathers rotate over the 4 SWDGE queues (4 Q7 core pairs in parallel),
  single_packet=True, <=1024 idx per call.
- BatchNorm statistics are AllReduced; conv bias folds into the BN shift.
- Graph pooling = free-dim window reduction; graph head + 128 per-node
  head MLPs run on the 64 local graphs.

Compute dtype: bf16 operands with fp32 PSUM/statistics.
"""

import os

import numpy as np
import ml_dtypes

import concourse.bacc as bacc
import concourse.tile as tile
import concourse.mybir as mybir
from concourse.bass_utils import run_bass_kernel_spmd

F32 = mybir.dt.float32
BF16 = mybir.dt.bfloat16
I16 = mybir.dt.int16
AF = mybir.ActivationFunctionType
OP = mybir.AluOpType

NBF = ml_dtypes.bfloat16

N = 65536
E = 524288
H = 128
B = 512
NPG = 128
NC = 8
NPC = N // NC      # 8192 nodes per core
HALF = N // 2
GPC = B // NC      # 64 graphs per core
EPS = 1e-5
NQ = 4             # SWDGE queues

K = 4              # tree slots per (node, half)
SG = 256           # spill group width (nodes)
NSG = NPC // SG    # 32 spill groups
SCALL = 1024       # idx per gather call
OCT = 1024         # nodes per "octet" (one slab call per slab)
NOCT = NPC // OCT  # 8


# ----------------------------------------------------------------- host prep

def _build_edge_plan(edge_index):
    src = edge_index[0].astype(np.int64)
    dst = edge_index[1].astype(np.int64)
    core = dst // NPC
    nloc = dst % NPC
    hh = (src >= HALF).astype(np.int64)
    soff = src - hh * HALF

    key = (core * 2 + hh) * NPC + nloc
    order = np.argsort(key, kind="stable")
    soff_s = soff[order]

    cnt = np.bincount(key[order], minlength=NC * 2 * NPC)
    starts = np.zeros(NC * 2 * NPC + 1, np.int64)
    np.cumsum(cnt, out=starts[1:])
    cnt3 = cnt.reshape(NC, 2, NPC)

    tok = (soff_s % 256) * 128 + soff_s // 256  # SBUF table token ids

    # slab stream layout per (core, half): oct-major:
    #   pos = oct*K*OCT + k*OCT + (n % OCT)
    idx_slab = np.zeros((NC, 2, NOCT, K, OCT), np.int16)
    negpad = np.zeros((NC, 2, NPC), np.float32)

    over = np.maximum(cnt3 - K, 0)
    spl_cnt = over.reshape(NC, 2, NSG, SG).sum(axis=3)
    s_ch = int(np.ceil(spl_cnt.max() / 128))
    sp_idx = np.zeros((NC, 2, NSG, s_ch * 128), np.int16)
    sp_drel = np.full((NC, 2, NSG, s_ch * 128), float(SG), np.float32)
    sp_fill = np.zeros((NC, 2, NSG), np.int64)

    for c in range(NC):
        for h in range(2):
            base = (c * 2 + h) * NPC
            for n in range(NPC):
                s0, s1 = starts[base + n], starts[base + n + 1]
                d = s1 - s0
                kk = min(d, K)
                o, nw = n // OCT, n % OCT
                idx_slab[c, h, o, :kk, nw] = tok[s0 : s0 + kk]
                negpad[c, h, n] = -(K - kk)
                if d > K:
                    g = n // SG
                    f = sp_fill[c, h, g]
                    m = d - K
                    # spill gathers read DRAM y_full rows: plain row ids
                    sp_idx[c, h, g, f : f + m] = soff_s[s0 + K : s1]
                    sp_drel[c, h, g, f : f + m] = n % SG
                    sp_fill[c, h, g] = f + m

    return s_ch, idx_slab, negpad, sp_idx, sp_drel


def _wrap_idx16(idx_flat):
    """[S] -> [128, S/16]: index i at partition i%16, col i//16, replicated
    over the 8 groups of 16 partitions."""
    s = idx_flat.shape[0]
    assert s % 16 == 0
    a = idx_flat.reshape(s // 16, 16).T
    return np.tile(a, (8, 1))


def _wrap_calls(flat, call):
    segs = [
        _wrap_idx16(flat[o : o + call]) for o in range(0, flat.shape[0], call)
    ]
    return np.concatenate(segs, axis=1)


# -------------------------------------------------------------- device build

def _sbuf_gather_nt(nc, out_ap, in_ap, idxs_ap, num_idxs, queue_num):
    """Non-transpose SBUF-source dma_gather. The bass helper only allows
    SBUF sources with transpose=True; the firmware decode supports the
    non-transpose combination (and it avoids the xbar, so multiple SWDGE
    queues can run concurrently). Construct the instruction directly."""
    eng = nc.gpsimd
    return eng.add_instruction(
        mybir.InstDMAGatherAnt(
            name=nc.get_next_instruction_name(),
            ins=[
                eng.lower_ap(in_ap),
                eng.lower_ap(idxs_ap),
                eng.lower_val_access(eng.to_reg(num_idxs)),
            ],
            outs=[eng.lower_ap(out_ap)],
            transpose=False,
            num_idxs=num_idxs,
            elem_size=128,
            stride_bytes_256=0,
            gen_mode=0,
            single_packet=True,
            queue_num=queue_num,
            sbuf_tokens_per_rank=128,
            sbuf_free_dim_per_rank=256,
            sbuf_free_dim_pad_per_rank=0,
            sbuf_byte_offset=0,
        )
    )


def _build(nc, s_ch):
    skip_gather = bool(int(os.environ.get("GNN_SKIP_GATHER", "0")))
    skip_cc = bool(int(os.environ.get("GNN_SKIP_CC", "0")))
    sp_per_h = NSG * s_ch * 128          # spill slots per half
    slab_cols = 2 * K * NPC // 16        # slab idx cols
    sp_cols = 2 * sp_per_h // 16
    sp_cpo = 4 * s_ch                    # spill chunks per oct (4 groups)

    def din(name, shape, dt):
        return nc.dram_tensor(name, shape, dt, kind="ExternalInput").ap()

    xin = din("xin", [32, NPC], BF16)
    idxs = din("idxs", [128, slab_cols], I16)
    spidx = din("spidx", [128, sp_cols], I16)
    drsp = din("drsp", [128, 2 * NSG * s_ch], BF16)
    iota = din("iota", [128, SG], BF16)
    ident = din("ident", [128, 128], BF16)
    npad = din("npad", [2, NPC], BF16)
    wn0 = din("wn0", [32, 128], BF16)
    wr0 = din("wr0", [32, 128], BF16)
    wn12 = din("wn12", [2, 128, 128], BF16)
    wr12 = din("wr12", [2, 128, 128], BF16)
    cb = din("cb", [128, 3], F32)
    bng = din("bng", [128, 3], F32)
    bnb = din("bnb", [128, 3], F32)
    gsw1 = din("gsw1", [128, 128], BF16)
    gsw2 = din("gsw2", [128, 128], BF16)
    ghw1 = din("ghw1", [128, 128], BF16)
    ghw2 = din("ghw2", [128, 64], BF16)
    ghw3 = din("ghw3", [64, 2], BF16)
    gsb1 = din("gsb1", [128, 1], F32)
    gsb2 = din("gsb2", [128, 1], F32)
    ghb1 = din("ghb1", [128, 1], F32)
    ghb2 = din("ghb2", [64, 1], F32)
    ghb3 = din("ghb3", [2, 1], F32)
    nhw1 = din("nhw1", [128, 128 * 128], BF16)
    nhw2 = din("nhw2", [128, 128 * 64], BF16)
    nhw3 = din("nhw3", [64, 128], BF16)
    nhb1 = din("nhb1", [128, 128], F32)
    nhb2 = din("nhb2", [64, 128], F32)
    nhb3 = din("nhb3", [1, 128], F32)

    outg = nc.dram_tensor("outg", [2, GPC], F32, kind="ExternalOutput").ap()
    outn = nc.dram_tensor("outn", [128, GPC], F32, kind="ExternalOutput").ap()

    y_local = nc.dram_tensor("y_local", [NPC, 128], BF16).ap()
    y_full = nc.dram_tensor("y_full", [N, 128], BF16, addr_space="Shared").ap()
    bn_in = [nc.dram_tensor(f"bn_in{l}", [128, 2], F32).ap() for l in range(3)]
    bn_out = [
        nc.dram_tensor(f"bn_out{l}", [128, 2], F32, addr_space="Shared").ap()
        for l in range(3)
    ]

    rg = [list(range(NC))]

    with tile.TileContext(nc) as tc:
        with (
            tc.tile_pool(name="persist", bufs=1) as pp,
            tc.tile_pool(name="small", bufs=2) as sp,
        ):
            # --- persistent tiles / constants
            x_bf = pp.tile([32, NPC], BF16)
            nc.sync.dma_start(out=x_bf[:], in_=xin)
            h_bf = pp.tile([128, NPC], BF16)
            h_raw = pp.tile([128, NPC], BF16)
            idx_sb = pp.tile([128, slab_cols], I16)
            nc.sync.dma_start(out=idx_sb[:], in_=idxs)
            spidx_sb = pp.tile([128, sp_cols], I16)
            nc.sync.dma_start(out=spidx_sb[:], in_=spidx)
            drsp_sb = pp.tile([128, 2 * NSG * s_ch], BF16)
            nc.sync.dma_start(out=drsp_sb[:], in_=drsp)
            iota_sb = pp.tile([128, SG], BF16)
            nc.sync.dma_start(out=iota_sb[:], in_=iota)
            id_sb = pp.tile([128, 128], BF16)
            nc.sync.dma_start(out=id_sb[:], in_=ident)
            npad_sb = pp.tile([2, NPC], BF16)
            nc.sync.dma_start(out=npad_sb[:], in_=npad)

            wn_sb = pp.tile([128, 3, 128], BF16)
            wr_sb = pp.tile([128, 3, 128], BF16)
            nc.sync.dma_start(out=wn_sb[:32, 0, :], in_=wn0)
            nc.sync.dma_start(out=wr_sb[:32, 0, :], in_=wr0)
            for l in range(2):
                nc.sync.dma_start(out=wn_sb[:, l + 1, :], in_=wn12[l])
                nc.sync.dma_start(out=wr_sb[:, l + 1, :], in_=wr12[l])
            cb_sb = pp.tile([128, 3], F32)
            nc.sync.dma_start(out=cb_sb[:], in_=cb)
            bng_sb = pp.tile([128, 3], F32)
            nc.sync.dma_start(out=bng_sb[:], in_=bng)
            bnb_sb = pp.tile([128, 3], F32)
            nc.sync.dma_start(out=bnb_sb[:], in_=bnb)

            # --- 3 GraphConv + BN + ReLU layers
            with (
                tc.tile_pool(name="ystage", bufs=1) as yp,
                tc.tile_pool(name="tab", bufs=1) as tbp,
                tc.tile_pool(name="agg", bufs=1) as agp,
                tc.tile_pool(name="mg", bufs=4) as mgp,
                tc.tile_pool(name="msp", bufs=4) as msppool,
                tc.tile_pool(name="ohsp", bufs=2) as ohp,
                tc.tile_pool(name="yr", bufs=2) as yrp,
                tc.tile_pool(name="sqp", bufs=1) as sqp,
                tc.tile_pool(name="psA", bufs=2, space="PSUM") as psA,
                tc.tile_pool(name="psR", bufs=3, space="PSUM") as psR,
            ):
                for l in range(3):
                    KIN = 32 if l == 0 else 128
                    hin = x_bf if l == 0 else h_bf
                    wn_l = wn_sb[:KIN, l, :]
                    wr_l = wr_sb[:KIN, l, :]

                    # A) y_local = (h^T @ wn) node-major bf16
                    ystage = yp.tile([128, NPC // 128, 128], BF16, tag="yst")
                    for blk in range(NPC // 128):
                        yps = psA.tile([128, 128], F32, space="PSUM", tag="yps")
                        nc.tensor.matmul(
                            out=yps[:],
                            lhsT=hin[:, blk * 128 : (blk + 1) * 128],
                            rhs=wn_l,
                            start=True,
                            stop=True,
                        )
                        nc.scalar.activation(
                            out=ystage[:, blk, :], in_=yps[:], func=AF.Copy
                        )
                    nc.sync.dma_start(
                        out=y_local.rearrange("(b p) f -> p b f", p=128),
                        in_=ystage[:],
                    )

                    # B) replicate the message table
                    if not skip_cc:
                        nc.gpsimd.collective_compute(
                            "AllGather",
                            OP.bypass,
                            replica_groups=rg,
                            ins=[y_local.opt()],
                            outs=[y_full.opt()],
                        )
                    # pad-correction rows y[0], y[HALF] (stacked, K=2)
                    yr2 = yrp.tile([2, 128], BF16, tag="yr2")
                    nc.sync.dma_start(out=yr2[0:1, :], in_=y_full[0:1, :])
                    nc.sync.dma_start(out=yr2[1:2, :], in_=y_full[HALF : HALF + 1, :])

                    # agg[node%128, node//128, feat], accumulated over slabs
                    agg = agp.tile([128, NPC // 128, 128], BF16, tag="agg")
                    qn = 0

                    def slab_call(h, o, k):
                        nonlocal qn
                        c0 = (h * K * NPC + o * K * OCT + k * OCT) // 16
                        mg = mgp.tile([128, OCT // 128, 128], BF16, tag="mg")
                        if not skip_gather:
                            _sbuf_gather_nt(
                                nc, mg[:], T[:],
                                idx_sb[:, c0 : c0 + OCT // 16],
                                OCT, qn % NQ,
                            )
                        qn += 1
                        ab = agg[:, o * (OCT // 128) : (o + 1) * (OCT // 128), :]
                        if h == 0 and k == 0:
                            nc.vector.tensor_copy(out=ab, in_=mg[:])
                        elif not skip_gather:
                            nc.vector.tensor_tensor(
                                out=ab, in0=ab, in1=mg[:], op=OP.add
                            )

                    def spill_call(h, o):
                        # split into sub-calls of <=8 chunks (1024 idx is the
                        # single_packet limit)
                        nonlocal qn
                        t = msppool.tile([128, sp_cpo, 128], BF16, tag="msp")
                        if not skip_gather:
                            for a in range(0, sp_cpo, 8):
                                b = min(a + 8, sp_cpo)
                                c0 = (h * sp_per_h + (o * sp_cpo + a) * 128) // 16
                                nc.gpsimd.dma_gather(
                                    t[:, a:b, :],
                                    y_full[:HALF] if h == 0 else y_full[HALF:],
                                    spidx_sb[:, c0 : c0 + (b - a) * 8],
                                    (b - a) * 128,
                                    (b - a) * 128,
                                    128,
                                    single_packet=True,
                                    queue_num=qn % NQ,
                                )
                                qn += 1
                        return t

                    # C) lo phase: table + slab gathers
                    T = tbp.tile([128, HALF // 128, 128], BF16, tag="T")
                    if not skip_gather:
                        nc.sync.dma_start(
                            out=T[:],
                            in_=y_full[:HALF].rearrange("(p s) f -> p s f", p=128),
                        )
                    for o in range(NOCT):
                        for k in range(K):
                            slab_call(0, o, k)
                    # lo-half spill gathers (from DRAM; independent of T) --
                    # these hide the T_hi reload on the gather stream
                    msp_lo = [spill_call(0, o) for o in range(2)]

                    # D) hi phase: table swap + slab gathers + spills + PSUM
                    T = tbp.tile([128, HALF // 128, 128], BF16, tag="T")
                    if not skip_gather:
                        nc.sync.dma_start(
                            out=T[:],
                            in_=y_full[HALF:].rearrange("(p s) f -> p s f", p=128),
                        )
                    for o in range(NOCT):
                        if o >= 2:
                            msp_lo.append(spill_call(0, o))
                        msp_hi = spill_call(1, o)
                        for k in range(K):
                            slab_call(1, o, k)
                        # PSUM groups for this oct (2 x 512 nodes)
                        for Gi in range(2):
                            G = o * 2 + Gi
                            pr_ = psR.tile(
                                [128, 512], F32, space="PSUM", tag="pr"
                            )
                            nc.tensor.matmul(
                                out=pr_[:],
                                lhsT=wr_l,
                                rhs=hin[:, G * 512 : (G + 1) * 512],
                                start=True,
                                stop=False,
                            )
                            nc.tensor.matmul(
                                out=pr_[:],
                                lhsT=yr2[:],
                                rhs=npad_sb[0:2, G * 512 : (G + 1) * 512],
                                start=False,
                                stop=False,
                            )
                            # spill one-hots + scatter, per half
                            n_ch = 2 * s_ch  # 2 groups x ch
                            for h in range(2):
                                oh = ohp.tile([128, n_ch, SG], BF16, tag="oh")
                                d0 = (h * NSG + 2 * G) * s_ch
                                nc.vector.tensor_tensor(
                                    out=oh[:],
                                    in0=iota_sb[:]
                                    .rearrange("p (c f) -> p c f", c=1)
                                    .to_broadcast([128, n_ch, SG]),
                                    in1=drsp_sb[:, d0 : d0 + n_ch]
                                    .rearrange("p (c f) -> p c f", f=1)
                                    .to_broadcast([128, n_ch, SG]),
                                    op=OP.is_equal,
                                )
                                if not skip_gather:
                                    for i in range(n_ch):
                                        g = 2 * G + i // s_ch
                                        ch = i % s_ch
                                        pos = (g - o * 4) * s_ch + ch
                                        t = msp_lo[o] if h == 0 else msp_hi
                                        nc.tensor.matmul(
                                            out=pr_[
                                                :,
                                                (g % 2) * SG : (g % 2 + 1) * SG,
                                            ],
                                            lhsT=t[:, pos, :],
                                            rhs=oh[:, i, :],
                                            start=False,
                                            stop=False,
                                        )
                            # agg transposes: 4 chunks of 128 nodes
                            for j in range(4):
                                nc.tensor.matmul(
                                    out=pr_[:, j * 128 : (j + 1) * 128],
                                    lhsT=agg[:, G * 4 + j, :],
                                    rhs=id_sb[:],
                                    start=False,
                                    stop=(j == 3),
                                )
                            nc.scalar.activation(
                                out=h_raw[:, G * 512 : (G + 1) * 512],
                                in_=pr_[:],
                                func=AF.Copy,
                            )

                    # E) BN statistics (sum, sumsq) + AllReduce
                    stats = sp.tile([128, 2], F32, tag="stats")
                    nc.vector.tensor_reduce(
                        out=stats[:, 0:1],
                        in_=h_raw[:],
                        axis=mybir.AxisListType.X,
                        op=OP.add,
                    )
                    sq = sqp.tile([128, 1024], F32, tag="sq")
                    s2p = sp.tile([128, 8], F32, tag="s2p")
                    for t in range(8):
                        nc.vector.tensor_tensor(
                            out=sq[:],
                            in0=h_raw[:, t * 1024 : (t + 1) * 1024],
                            in1=h_raw[:, t * 1024 : (t + 1) * 1024],
                            op=OP.mult,
                        )
                        nc.vector.tensor_reduce(
                            out=s2p[:, t : t + 1],
                            in_=sq[:],
                            axis=mybir.AxisListType.X,
                            op=OP.add,
                        )
                    nc.vector.tensor_reduce(
                        out=stats[:, 1:2],
                        in_=s2p[:],
                        axis=mybir.AxisListType.X,
                        op=OP.add,
                    )
                    nc.sync.dma_start(out=bn_in[l], in_=stats[:])
                    nc.gpsimd.collective_compute(
                        "AllReduce",
                        OP.add,
                        replica_groups=rg,
                        ins=[bn_in[l].opt()],
                        outs=[bn_out[l].opt()],
                    )
                    gstats = sp.tile([128, 2], F32, tag="gstats")
                    nc.sync.dma_start(out=gstats[:], in_=bn_out[l])

                    # F) scale/shift: m = s1/N + cb; v = s2/N - (s1/N)^2
                    pr = sp.tile([128, 6], F32, tag="bnpar")
                    nc.vector.tensor_scalar_mul(pr[:, 0:1], gstats[:, 0:1], 1.0 / N)
                    nc.vector.tensor_scalar_mul(pr[:, 1:2], gstats[:, 1:2], 1.0 / N)
                    nc.vector.tensor_tensor(
                        out=pr[:, 2:3], in0=pr[:, 0:1], in1=pr[:, 0:1], op=OP.mult
                    )
                    nc.vector.tensor_tensor(
                        out=pr[:, 1:2], in0=pr[:, 1:2], in1=pr[:, 2:3],
                        op=OP.subtract,
                    )
                    nc.vector.tensor_scalar_add(pr[:, 1:2], pr[:, 1:2], EPS)
                    nc.scalar.sqrt(out=pr[:, 2:3], in_=pr[:, 1:2])
                    nc.vector.reciprocal(out=pr[:, 3:4], in_=pr[:, 2:3])
                    nc.vector.tensor_tensor(
                        out=pr[:, 3:4], in0=pr[:, 3:4],
                        in1=bng_sb[:, l : l + 1], op=OP.mult,
                    )
                    nc.vector.tensor_tensor(
                        out=pr[:, 0:1], in0=pr[:, 0:1],
                        in1=cb_sb[:, l : l + 1], op=OP.add,
                    )
                    nc.vector.tensor_tensor(
                        out=pr[:, 4:5], in0=pr[:, 0:1], in1=pr[:, 3:4], op=OP.mult
                    )
                    nc.vector.tensor_tensor(
                        out=pr[:, 5:6], in0=bnb_sb[:, l : l + 1],
                        in1=pr[:, 4:5], op=OP.subtract,
                    )

                    # G) h = relu(h_raw * scale + shift), bf16
                    for t in range(4):
                        nc.scalar.activation(
                            out=h_bf[:, t * 2048 : (t + 1) * 2048],
                            in_=h_raw[:, t * 2048 : (t + 1) * 2048],
                            func=AF.Relu,
                            bias=pr[:, 5:6],
                            scale=pr[:, 3:4],
                        )

            # --- heads (layers-scope pools are closed; SBUF freed)
            with (
                tc.tile_pool(name="hw", bufs=1) as hwp,
                tc.tile_pool(name="hsb", bufs=3) as hsb,
                tc.tile_pool(name="hps", bufs=2, space="PSUM") as hps,
                tc.tile_pool(name="hcst", bufs=1) as hc,
            ):
                # prefetch all node-head weights up front (SBUF is free now)
                PCHUNK = 16
                w1t, w2t = [], []
                for pc in range(NPG // PCHUNK):
                    w1 = hwp.tile([128, PCHUNK * 128], BF16, tag=f"w1_{pc}")
                    nc.sync.dma_start(
                        out=w1[:],
                        in_=nhw1[:, pc * PCHUNK * 128 : (pc + 1) * PCHUNK * 128],
                    )
                    w1t.append(w1)
                    w2 = hwp.tile([128, PCHUNK * 64], BF16, tag=f"w2_{pc}")
                    nc.sync.dma_start(
                        out=w2[:],
                        in_=nhw2[:, pc * PCHUNK * 64 : (pc + 1) * PCHUNK * 64],
                    )
                    w2t.append(w2)

                # graph pooling (contiguous 128-node graphs) + graph head
                pool = hc.tile([128, GPC], F32)
                nc.vector.tensor_reduce(
                    out=pool[:],
                    in_=h_bf[:].rearrange("p (g n) -> p g n", g=GPC),
                    axis=mybir.AxisListType.X,
                    op=OP.add,
                )
                gw = {}
                for name, apw, shape in (
                    ("gsw1", gsw1, [128, 128]),
                    ("gsw2", gsw2, [128, 128]),
                    ("ghw1", ghw1, [128, 128]),
                    ("ghw2", ghw2, [128, 64]),
                    ("ghw3", ghw3, [64, 2]),
                ):
                    t = hc.tile(shape, BF16, tag=name)
                    nc.sync.dma_start(out=t[:], in_=apw)
                    gw[name] = t
                gb = {}
                for name, apb, p in (
                    ("gsb1", gsb1, 128),
                    ("gsb2", gsb2, 128),
                    ("ghb1", ghb1, 128),
                    ("ghb2", ghb2, 64),
                    ("ghb3", ghb3, 2),
                ):
                    t = hc.tile([p, 1], F32, tag=name)
                    nc.sync.dma_start(out=t[:], in_=apb)
                    gb[name] = t

                g0 = hsb.tile([128, GPC], BF16, tag="g0")
                nc.scalar.activation(
                    out=g0[:], in_=pool[:], func=AF.Relu, scale=1.0 / NPG
                )
                gp1 = hps.tile([128, GPC], F32, space="PSUM", tag="gps")
                nc.tensor.matmul(
                    out=gp1[:], lhsT=gw["gsw1"][:], rhs=g0[:], start=True, stop=True
                )
                g1 = hsb.tile([128, GPC], BF16, tag="g1")
                nc.vector.tensor_scalar_add(g1[:], gp1[:], gb["gsb1"][:])
                gp2 = hps.tile([128, GPC], F32, space="PSUM", tag="gps")
                nc.tensor.matmul(
                    out=gp2[:], lhsT=gw["gsw2"][:], rhs=g1[:], start=True, stop=True
                )
                g2 = hsb.tile([128, GPC], BF16, tag="g2")
                nc.scalar.activation(
                    out=g2[:], in_=gp2[:], func=AF.Relu, bias=gb["gsb2"][:]
                )
                gp3 = hps.tile([128, GPC], F32, space="PSUM", tag="gps")
                nc.tensor.matmul(
                    out=gp3[:], lhsT=gw["ghw1"][:], rhs=g2[:], start=True, stop=True
                )
                g3 = hsb.tile([128, GPC], BF16, tag="g3")
                nc.scalar.activation(
                    out=g3[:], in_=gp3[:], func=AF.Relu, bias=gb["ghb1"][:]
                )
                gp4 = hps.tile([64, GPC], F32, space="PSUM", tag="gps")
                nc.tensor.matmul(
                    out=gp4[:], lhsT=gw["ghw2"][:], rhs=g3[:], start=True, stop=True
                )
                g4 = hsb.tile([64, GPC], BF16, tag="g4")
                nc.scalar.activation(
                    out=g4[:], in_=gp4[:], func=AF.Relu, bias=gb["ghb2"][:]
                )
                gp5 = hps.tile([2, GPC], F32, space="PSUM", tag="gps")
                nc.tensor.matmul(
                    out=gp5[:], lhsT=gw["ghw3"][:], rhs=g4[:], start=True, stop=True
                )
                gout = hsb.tile([2, GPC], F32, tag="gout")
                nc.vector.tensor_scalar_add(gout[:], gp5[:], gb["ghb3"][:])
                nc.sync.dma_start(out=outg, in_=gout[:])

                # node heads: 128 positions x (128->128->64->1), 64 graphs each
                nb1 = hc.tile([128, 128], F32, tag="nb1")
                nc.sync.dma_start(out=nb1[:], in_=nhb1)
                nb2 = hc.tile([64, 128], F32, tag="nb2")
                nc.sync.dma_start(out=nb2[:], in_=nhb2)
                nb3 = hc.tile([1, 128], F32, tag="nb3")
                nc.sync.dma_start(out=nb3[:], in_=nhb3)
                w3 = hc.tile([64, 128], BF16, tag="w3")
                nc.sync.dma_start(out=w3[:], in_=nhw3)
                # single-partition accumulator: engines can't write at a
                # nonzero partition offset, so row p lives at cols [p*GPC,...)
                out_n = hc.tile([1, NPG * GPC], F32, tag="out_n")

                for pc in range(NPG // PCHUNK):
                    w1, w2 = w1t[pc], w2t[pc]
                    for pi in range(PCHUNK):
                        p = pc * PCHUNK + pi
                        zp1 = hps.tile([128, GPC], F32, space="PSUM", tag="zp1")
                        nc.tensor.matmul(
                            out=zp1[:],
                            lhsT=w1[:, pi * 128 : (pi + 1) * 128],
                            rhs=h_bf[:, p :: NPG],
                            start=True,
                            stop=True,
                        )
                        z1 = hsb.tile([128, GPC], BF16, tag="z1")
                        nc.scalar.activation(
                            out=z1[:], in_=zp1[:], func=AF.Relu,
                            bias=nb1[:, p : p + 1],
                        )
                        zp2 = hps.tile([64, GPC], F32, space="PSUM", tag="zp2")
                        nc.tensor.matmul(
                            out=zp2[:],
                            lhsT=w2[:, pi * 64 : (pi + 1) * 64],
                            rhs=z1[:],
                            start=True,
                            stop=True,
                        )
                        z2 = hsb.tile([64, GPC], BF16, tag="z2")
                        nc.scalar.activation(
                            out=z2[:], in_=zp2[:], func=AF.Relu,
                            bias=nb2[:, p : p + 1],
                        )
                        zp3 = hps.tile([1, GPC], F32, space="PSUM", tag="zp3")
                        nc.tensor.matmul(
                            out=zp3[:],
                            lhsT=w3[:, p : p + 1],
                            rhs=z2[:],
                            start=True,
                            stop=True,
                        )
                        nc.vector.tensor_scalar_add(
                            out_n[:, p * GPC : (p + 1) * GPC],
                            zp3[:],
                            nb3[:, p : p + 1],
                        )
                nc.sync.dma_start(
                    out=outn.rearrange("(o p) g -> o (p g)", o=1), in_=out_n[:]
                )

    nc.compile()


# ------------------------------------------------------------------- driver

def _prep_inputs(inputs):
    f32 = lambda k: np.asarray(inputs[k], np.float32)
    bf16 = lambda a: np.ascontiguousarray(a).astype(NBF)

    edge_index = np.asarray(inputs["edge_index"], np.int64)
    s_ch, idx_slab, negpad, sp_idx, sp_drel = _build_edge_plan(edge_index)

    x = f32("x")
    iota = np.tile(np.arange(SG, dtype=np.float32), (128, 1))

    shared = {
        "iota": bf16(iota),
        "ident": bf16(np.eye(128, dtype=np.float32)),
        "wn0": bf16(f32("conv0_wn")),
        "wr0": bf16(f32("conv0_wr")),
        "wn12": bf16(f32("convs_wn")),
        "wr12": bf16(f32("convs_wr")),
        "cb": np.stack(
            [f32("conv0_b"), f32("convs_b")[0], f32("convs_b")[1]], axis=1
        ).copy(),
        "bng": np.stack(
            [f32("bn0_g"), f32("bns_g")[0], f32("bns_g")[1]], axis=1
        ).copy(),
        "bnb": np.stack(
            [f32("bn0_b"), f32("bns_b")[0], f32("bns_b")[1]], axis=1
        ).copy(),
        "gsw1": bf16(f32("gs_w1")),
        "gsw2": bf16(f32("gs_w2")),
        "ghw1": bf16(f32("gh_w1")),
        "ghw2": bf16(f32("gh_w2")),
        "ghw3": bf16(f32("gh_w3")),
        "gsb1": f32("gs_b1").reshape(128, 1).copy(),
        "gsb2": f32("gs_b2").reshape(128, 1).copy(),
        "ghb1": f32("gh_b1").reshape(128, 1).copy(),
        "ghb2": f32("gh_b2").reshape(64, 1).copy(),
        "ghb3": f32("gh_b3").reshape(2, 1).copy(),
        "nhw1": bf16(f32("nh_w1").transpose(1, 0, 2).reshape(128, 128 * 128)),
        "nhw2": bf16(f32("nh_w2").transpose(1, 0, 2).reshape(128, 128 * 64)),
        "nhw3": bf16(f32("nh_w3")[:, :, 0].T),
        "nhb1": f32("nh_b1").T.copy(),
        "nhb2": f32("nh_b2").T.copy(),
        "nhb3": f32("nh_b3").T.copy(),
    }

    in_maps = []
    for c in range(NC):
        idx_w = _wrap_calls(idx_slab[c].reshape(-1), OCT).copy()
        sp_w = _wrap_calls(sp_idx[c].reshape(-1), SCALL).copy()
        # spill dstrel, chunk-major columns: col = (h*NSG + g)*s_ch + ch
        dr = sp_drel[c].reshape(-1, 128).T
        in_maps.append(
            dict(
                shared,
                xin=bf16(x[c * NPC : (c + 1) * NPC].T),
                idxs=idx_w,
                spidx=sp_w,
                drsp=dr.astype(NBF).copy(),
                npad=negpad[c].astype(NBF).copy(),
            )
        )
    return s_ch, in_maps


def _numpy_fallback(inputs):
    """Reference math in numpy for unexpected input layouts."""
    f = lambda k: np.asarray(inputs[k], np.float32)
    x = f("x")
    src, dst = np.asarray(inputs["edge_index"], np.int64)
    batch = np.asarray(inputs["batch"], np.int64)

    def gconv(h, wr, wn, b):
        y = h @ wn
        agg = np.zeros_like(h @ wr)
        np.add.at(agg, dst, y[src])
        return h @ wr + agg + b

    def bn(h, g, bt):
        m = h.mean(0)
        v = h.var(0)
        return (h - m) / np.sqrt(v + EPS) * g + bt

    h = np.maximum(bn(gconv(x, f("conv0_wr"), f("conv0_wn"), f("conv0_b")),
                      f("bn0_g"), f("bn0_b")), 0)
    for i in range(2):
        h = np.maximum(
            bn(gconv(h, f("convs_wr")[i], f("convs_wn")[i], f("convs_b")[i]),
               f("bns_g")[i], f("bns_b")[i]), 0)
    counts = np.bincount(batch, minlength=B).astype(np.float32)
    xg = np.zeros((B, H), np.float32)
    np.add.at(xg, batch, h)
    xg /= counts[:, None]
    g = np.maximum(xg, 0)
    g = g @ f("gs_w1") + f("gs_b1")
    g = np.maximum(g @ f("gs_w2") + f("gs_b2"), 0)
    g = np.maximum(g @ f("gh_w1") + f("gh_b1"), 0)
    g = np.maximum(g @ f("gh_w2") + f("gh_b2"), 0)
    g = g @ f("gh_w3") + f("gh_b3")
    xn = h.reshape(B, NPG, H)
    z = np.maximum(np.einsum("bnf,nfh->bnh", xn, f("nh_w1")) + f("nh_b1"), 0)
    z = np.maximum(np.einsum("bnh,nhk->bnk", z, f("nh_w2")) + f("nh_b2"), 0)
    z = np.einsum("bnk,nko->bno", z, f("nh_w3")) + f("nh_b3")
    return np.concatenate([g, z[:, :, 0]], axis=1).astype(np.float32)


def _run(inputs, trace=False, trace_kwargs=None):
    batch = np.asarray(inputs["batch"], np.int64)
    if not (
        np.array_equal(batch, np.arange(N, dtype=np.int64) // NPG)
        and np.asarray(inputs["x"]).shape == (N, 32)
        and np.asarray(inputs["edge_index"]).shape == (2, E)
    ):
        return _numpy_fallback(inputs), None

    s_ch, in_maps = _prep_inputs(inputs)
    nc = bacc.Bacc(
        "TRN2",
        target_bir_lowering=False,
        debug=False,
        num_devices=NC,
        num_swdge_queues=NQ,
    )
    _build(nc, s_ch)
    r = run_bass_kernel_spmd(
        nc, in_maps, list(range(NC)), trace=trace, **(trace_kwargs or {})
    )
    out = np.zeros((B, 2 + NPG), np.float32)
    for c in range(NC):
        out[c * GPC : (c + 1) * GPC, 0:2] = r.results[c]["outg"].T
        out[c * GPC : (c + 1) * GPC, 2:] = r.results[c]["outn"].T
    return out, r


def kernel(**inputs):
    out, _ = _run(inputs)
    return out


# revision 22
# speedup vs baseline: 1.3590x; 1.3590x over previous
"""GNN message-passing kernel for 8 TRN2 NeuronCores (Bass/Tile, SPMD).

Takes the FULL inputs of nn_Base_40793599378196 and returns the FULL
[512, 130] output. Internally:

- Nodes/graphs sharded by graph: core c owns nodes [c*8192, (c+1)*8192).
  Weights replicated. Per layer y = h @ wn is computed locally (node-major
  bf16), AllGathered into a full [65536, 128] DRAM table.
- Aggregation avoids both the random-256B-HBM-read drain floor (~105ns per
  descriptor per SDMA engine) and the one-hot/scatter-matmul volume:
  * Each 32768-node half of the table is DMA'd contiguously into an SBUF
    table (8MB, token t = y row (t%128)*256 + t//128 at partition t%128,
    stripe t//128 -- so the idx for row r is (r%256)*128 + r//256).
  * Edges are organized per (dst node, src half) into K=4 fixed slots;
    slot-slab streams are gathered EDGE-MAJOR with a non-transpose
    SBUF-source dma_gather (constructed directly -- the bass helper only
    allows SBUF sources with transpose=True, but the firmware supports
    non-transpose too, avoiding the xbar so the 4 SWDGE queues can
    generate descriptors concurrently; SBUF reads make the drain cheap).
  * The K slabs accumulate into agg[node%128, node//128, feat] with plain
    DVE adds; per 512-node group a transpose-matmul (agg_chunk^T @ I)
    accumulates agg into the root-transform PSUM group. Pad slots gather
    token 0 and are cancelled by a rank-1 correction matmul.
  * Overflow edges (degree > K per half, ~2.5%) are gathered from DRAM
    y_full (edge-major chunks per 256-node spill group) and scattered via
    one-hot (iota256 is_equal) matmuls into the same PSUM groups.
- Gathers rotate over the 4 SWDGE queues (4 Q7 core pairs in parallel),
  single_packet=True, <=1024 idx per call.
- BatchNorm statistics are AllReduced; conv bias folds into the BN shift.
- Graph pooling = free-dim window reduction; graph head + 128 per-node
  head MLPs run on the 64 local graphs.

Compute dtype: bf16 operands with fp32 PSUM/statistics.
"""

import os

import numpy as np
import ml_dtypes

import concourse.bacc as bacc
import concourse.tile as tile
import concourse.mybir as mybir
from concourse.bass_utils import run_bass_kernel_spmd

F32 = mybir.dt.float32
BF16 = mybir.dt.bfloat16
I16 = mybir.dt.int16
AF = mybir.ActivationFunctionType
OP = mybir.AluOpType

NBF = ml_dtypes.bfloat16

N = 65536
E = 524288
H = 128
B = 512
NPG = 128
NC = 8
NPC = N // NC      # 8192 nodes per core
HALF = N // 2
GPC = B // NC      # 64 graphs per core
EPS = 1e-5
NQ = 4             # SWDGE queues

K = 3              # tree slots per (node, half)
SG = 256           # spill group width (nodes)
NSG = NPC // SG    # 32 spill groups
SCALL = 1024       # idx per gather call
OCT = 1024         # nodes per "octet" (one slab call per slab)
NOCT = NPC // OCT  # 8


# ----------------------------------------------------------------- host prep

def _build_edge_plan(edge_index):
    src = edge_index[0].astype(np.int64)
    dst = edge_index[1].astype(np.int64)
    core = dst // NPC
    nloc = dst % NPC
    hh = (src >= HALF).astype(np.int64)
    soff = src - hh * HALF

    key = (core * 2 + hh) * NPC + nloc
    order = np.argsort(key, kind="stable")
    soff_s = soff[order]

    cnt = np.bincount(key[order], minlength=NC * 2 * NPC)
    starts = np.zeros(NC * 2 * NPC + 1, np.int64)
    np.cumsum(cnt, out=starts[1:])
    cnt3 = cnt.reshape(NC, 2, NPC)

    tok = (soff_s % 256) * 128 + soff_s // 256  # SBUF table token ids

    # slab stream layout per (core, half): oct-major:
    #   pos = oct*K*OCT + k*OCT + (n % OCT)
    idx_slab = np.zeros((NC, 2, NOCT, K, OCT), np.int16)
    negpad = np.zeros((NC, 2, NPC), np.float32)

    over = np.maximum(cnt3 - K, 0)
    spl_cnt = over.reshape(NC, 2, NSG, SG).sum(axis=3)
    s_ch = int(np.ceil(spl_cnt.max() / 128))
    sp_idx = np.zeros((NC, 2, NSG, s_ch * 128), np.int16)
    sp_drel = np.full((NC, 2, NSG, s_ch * 128), float(SG), np.float32)
    sp_fill = np.zeros((NC, 2, NSG), np.int64)

    for c in range(NC):
        for h in range(2):
            base = (c * 2 + h) * NPC
            for n in range(NPC):
                s0, s1 = starts[base + n], starts[base + n + 1]
                d = s1 - s0
                kk = min(d, K)
                o, nw = n // OCT, n % OCT
                idx_slab[c, h, o, :kk, nw] = tok[s0 : s0 + kk]
                negpad[c, h, n] = -(K - kk)
                if d > K:
                    g = n // SG
                    f = sp_fill[c, h, g]
                    m = d - K
                    # lo spills gather DRAM y_full rows (so they can run in
                    # the hi phase); hi spills gather the SBUF table (tokens)
                    vals = soff_s[s0 + K : s1] if h == 0 else tok[s0 + K : s1]
                    sp_idx[c, h, g, f : f + m] = vals
                    sp_drel[c, h, g, f : f + m] = n % SG
                    sp_fill[c, h, g] = f + m

    return s_ch, idx_slab, negpad, sp_idx, sp_drel


def _wrap_idx16(idx_flat):
    """[S] -> [128, S/16]: index i at partition i%16, col i//16, replicated
    over the 8 groups of 16 partitions."""
    s = idx_flat.shape[0]
    assert s % 16 == 0
    a = idx_flat.reshape(s // 16, 16).T
    return np.tile(a, (8, 1))


def _wrap_calls(flat, call):
    segs = [
        _wrap_idx16(flat[o : o + call]) for o in range(0, flat.shape[0], call)
    ]
    return np.concatenate(segs, axis=1)


# -------------------------------------------------------------- device build

def _sbuf_gather_nt(nc, out_ap, in_ap, idxs_ap, num_idxs, queue_num):
    """Non-transpose SBUF-source dma_gather. The bass helper only allows
    SBUF sources with transpose=True; the firmware decode supports the
    non-transpose combination (and it avoids the xbar, so multiple SWDGE
    queues can run concurrently). Construct the instruction directly."""
    eng = nc.gpsimd
    return eng.add_instruction(
        mybir.InstDMAGatherAnt(
            name=nc.get_next_instruction_name(),
            ins=[
                eng.lower_ap(in_ap),
                eng.lower_ap(idxs_ap),
                eng.lower_val_access(eng.to_reg(num_idxs)),
            ],
            outs=[eng.lower_ap(out_ap)],
            transpose=False,
            num_idxs=num_idxs,
            elem_size=128,
            stride_bytes_256=0,
            gen_mode=0,
            single_packet=True,
            queue_num=queue_num,
            sbuf_tokens_per_rank=128,
            sbuf_free_dim_per_rank=256,
            sbuf_free_dim_pad_per_rank=0,
            sbuf_byte_offset=0,
        )
    )


def _build(nc, s_ch):
    skip_gather = bool(int(os.environ.get("GNN_SKIP_GATHER", "0")))
    skip_cc = bool(int(os.environ.get("GNN_SKIP_CC", "0")))
    sp_per_h = NSG * s_ch * 128          # spill slots per half
    slab_cols = 2 * K * NPC // 16        # slab idx cols
    sp_cols = 2 * sp_per_h // 16
    sp_cpo = 4 * s_ch                    # spill chunks per oct (4 groups)

    def din(name, shape, dt):
        return nc.dram_tensor(name, shape, dt, kind="ExternalInput").ap()

    xin = din("xin", [32, NPC], BF16)
    idxs = din("idxs", [128, slab_cols], I16)
    spidx = din("spidx", [128, sp_cols], I16)
    drsp = din("drsp", [128, 2 * NSG * s_ch], BF16)
    iota = din("iota", [128, SG], BF16)
    ident = din("ident", [128, 128], BF16)
    npad = din("npad", [2, NPC], BF16)
    wn0 = din("wn0", [32, 128], BF16)
    wr0 = din("wr0", [32, 128], BF16)
    wn12 = din("wn12", [2, 128, 128], BF16)
    wr12 = din("wr12", [2, 128, 128], BF16)
    cb = din("cb", [128, 3], F32)
    bng = din("bng", [128, 3], F32)
    bnb = din("bnb", [128, 3], F32)
    gsw1 = din("gsw1", [128, 128], BF16)
    gsw2 = din("gsw2", [128, 128], BF16)
    ghw1 = din("ghw1", [128, 128], BF16)
    ghw2 = din("ghw2", [128, 64], BF16)
    ghw3 = din("ghw3", [64, 2], BF16)
    gsb1 = din("gsb1", [128, 1], F32)
    gsb2 = din("gsb2", [128, 1], F32)
    ghb1 = din("ghb1", [128, 1], F32)
    ghb2 = din("ghb2", [64, 1], F32)
    ghb3 = din("ghb3", [2, 1], F32)
    nhw1 = din("nhw1", [128, 128 * 128], BF16)
    nhw2 = din("nhw2", [128, 128 * 64], BF16)
    nhw3 = din("nhw3", [64, 128], BF16)
    nhb1 = din("nhb1", [128, 128], F32)
    nhb2 = din("nhb2", [64, 128], F32)
    nhb3 = din("nhb3", [1, 128], F32)

    outg = nc.dram_tensor("outg", [2, GPC], F32, kind="ExternalOutput").ap()
    outn = nc.dram_tensor("outn", [128, GPC], F32, kind="ExternalOutput").ap()

    y_local = nc.dram_tensor("y_local", [NPC, 128], BF16).ap()
    y_full = nc.dram_tensor("y_full", [N, 128], BF16, addr_space="Shared").ap()
    bn_in = [nc.dram_tensor(f"bn_in{l}", [128, 2], F32).ap() for l in range(3)]
    bn_out = [
        nc.dram_tensor(f"bn_out{l}", [128, 2], F32, addr_space="Shared").ap()
        for l in range(3)
    ]

    rg = [list(range(NC))]

    with tile.TileContext(nc) as tc:
        with (
            tc.tile_pool(name="persist", bufs=1) as pp,
            tc.tile_pool(name="small", bufs=2) as sp,
        ):
            # --- persistent tiles / constants
            x_bf = pp.tile([32, NPC], BF16)
            nc.sync.dma_start(out=x_bf[:], in_=xin)
            h_bf = pp.tile([128, NPC], BF16)
            h_raw = pp.tile([128, NPC], F32)
            idx_sb = pp.tile([128, slab_cols], I16)
            nc.sync.dma_start(out=idx_sb[:], in_=idxs)
            spidx_sb = pp.tile([128, sp_cols], I16)
            nc.sync.dma_start(out=spidx_sb[:], in_=spidx)
            drsp_sb = pp.tile([128, 2 * NSG * s_ch], BF16)
            nc.sync.dma_start(out=drsp_sb[:], in_=drsp)
            iota_sb = pp.tile([128, SG], BF16)
            nc.sync.dma_start(out=iota_sb[:], in_=iota)
            id_sb = pp.tile([128, 128], BF16)
            nc.sync.dma_start(out=id_sb[:], in_=ident)
            npad_sb = pp.tile([2, NPC], BF16)
            nc.sync.dma_start(out=npad_sb[:], in_=npad)

            wn_sb = pp.tile([128, 3, 128], BF16)
            wr_sb = pp.tile([128, 3, 128], BF16)
            nc.sync.dma_start(out=wn_sb[:32, 0, :], in_=wn0)
            nc.sync.dma_start(out=wr_sb[:32, 0, :], in_=wr0)
            for l in range(2):
                nc.sync.dma_start(out=wn_sb[:, l + 1, :], in_=wn12[l])
                nc.sync.dma_start(out=wr_sb[:, l + 1, :], in_=wr12[l])
            cb_sb = pp.tile([128, 3], F32)
            nc.sync.dma_start(out=cb_sb[:], in_=cb)
            bng_sb = pp.tile([128, 3], F32)
            nc.sync.dma_start(out=bng_sb[:], in_=bng)
            bnb_sb = pp.tile([128, 3], F32)
            nc.sync.dma_start(out=bnb_sb[:], in_=bnb)

            # --- 3 GraphConv + BN + ReLU layers
            with (
                tc.tile_pool(name="ystage", bufs=1) as yp,
                tc.tile_pool(name="tab", bufs=1) as tbp,
                tc.tile_pool(name="mg", bufs=3) as mgp,
                tc.tile_pool(name="msp", bufs=3) as msppool,
                tc.tile_pool(name="ohsp", bufs=2) as ohp,
                tc.tile_pool(name="yr", bufs=2) as yrp,
                tc.tile_pool(name="sqp", bufs=1) as sqp,
                tc.tile_pool(name="psA", bufs=2, space="PSUM") as psA,
                tc.tile_pool(name="psR", bufs=3, space="PSUM") as psR,
            ):
                for l in range(3):
                    KIN = 32 if l == 0 else 128
                    hin = x_bf if l == 0 else h_bf
                    wn_l = wn_sb[:KIN, l, :]
                    wr_l = wr_sb[:KIN, l, :]

                    # A) y_local = (h^T @ wn) node-major bf16
                    # (shares the buffer with agg: disjoint lifetimes)
                    ystage = yp.tile([128, NPC // 128, 128], BF16, tag="stg")
                    for blk in range(NPC // 128):
                        yps = psA.tile([128, 128], F32, space="PSUM", tag="yps")
                        nc.tensor.matmul(
                            out=yps[:],
                            lhsT=hin[:, blk * 128 : (blk + 1) * 128],
                            rhs=wn_l,
                            start=True,
                            stop=True,
                        )
                        nc.scalar.activation(
                            out=ystage[:, blk, :], in_=yps[:], func=AF.Copy
                        )
                    nc.sync.dma_start(
                        out=y_local.rearrange("(b p) f -> p b f", p=128),
                        in_=ystage[:],
                    )

                    # B) replicate the message table
                    if not skip_cc:
                        nc.gpsimd.collective_compute(
                            "AllGather",
                            OP.bypass,
                            replica_groups=rg,
                            ins=[y_local.opt()],
                            outs=[y_full.opt()],
                        )
                    # pad-correction rows y[0], y[HALF] (stacked, K=2)
                    yr2 = yrp.tile([2, 128], BF16, tag="yr2")
                    nc.sync.dma_start(out=yr2[0:1, :], in_=y_full[0:1, :])
                    nc.sync.dma_start(out=yr2[1:2, :], in_=y_full[HALF : HALF + 1, :])

                    # agg[node%128, node//128, feat], accumulated over slabs
                    # (same pool/tag as ystage: reuses its buffer)
                    agg = yp.tile([128, NPC // 128, 128], BF16, tag="stg")
                    qn = 0

                    def slab_call(h, o, k):
                        nonlocal qn
                        c0 = (h * K * NPC + o * K * OCT + k * OCT) // 16
                        mg = mgp.tile([128, OCT // 128, 128], BF16, tag="mg")
                        if not skip_gather:
                            _sbuf_gather_nt(
                                nc, mg[:], T[:],
                                idx_sb[:, c0 : c0 + OCT // 16],
                                OCT, qn % NQ,
                            )
                        qn += 1
                        ab = agg[:, o * (OCT // 128) : (o + 1) * (OCT // 128), :]
                        if h == 0 and k == 0:
                            nc.vector.tensor_copy(out=ab, in_=mg[:])
                        elif not skip_gather:
                            nc.vector.tensor_tensor(
                                out=ab, in0=ab, in1=mg[:], op=OP.add
                            )

                    def spill_call(h, o):
                        # sub-calls of <=8 chunks (1024-idx single_packet cap)
                        # h==0: DRAM rows (usable in the hi phase); h==1: SBUF
                        # table tokens (cheap drains)
                        nonlocal qn
                        t = msppool.tile([128, sp_cpo, 128], BF16, tag="msp")
                        if not skip_gather:
                            for a in range(0, sp_cpo, 8):
                                b = min(a + 8, sp_cpo)
                                c0 = (h * sp_per_h + (o * sp_cpo + a) * 128) // 16
                                if h == 0:
                                    nc.gpsimd.dma_gather(
                                        t[:, a:b, :],
                                        y_full[:HALF],
                                        spidx_sb[:, c0 : c0 + (b - a) * 8],
                                        (b - a) * 128,
                                        (b - a) * 128,
                                        128,
                                        single_packet=True,
                                        queue_num=qn % NQ,
                                    )
                                else:
                                    _sbuf_gather_nt(
                                        nc,
                                        t[:, a:b, :],
                                        T[:],
                                        spidx_sb[:, c0 : c0 + (b - a) * 8],
                                        (b - a) * 128,
                                        qn % NQ,
                                    )
                                qn += 1
                        return t

                    # C) lo phase: table + slab gathers
                    T = tbp.tile([128, HALF // 128, 128], BF16, tag="T")
                    if not skip_gather:
                        nc.sync.dma_start(
                            out=T[:],
                            in_=y_full[:HALF].rearrange("(p s) f -> p s f", p=128),
                        )
                    for o in range(NOCT):
                        for k in range(K):
                            slab_call(0, o, k)

                    # D) hi phase: table swap + slab gathers + spills + PSUM
                    T = tbp.tile([128, HALF // 128, 128], BF16, tag="T")
                    if not skip_gather:
                        nc.sync.dma_start(
                            out=T[:],
                            in_=y_full[HALF:].rearrange("(p s) f -> p s f", p=128),
                        )
                    for o in range(NOCT):
                        msp_lo_t = spill_call(0, o)
                        msp_hi = spill_call(1, o)
                        for k in range(K):
                            slab_call(1, o, k)
                        # PSUM groups for this oct (2 x 512 nodes)
                        for Gi in range(2):
                            G = o * 2 + Gi
                            pr_ = psR.tile(
                                [128, 512], F32, space="PSUM", tag="pr"
                            )
                            nc.tensor.matmul(
                                out=pr_[:],
                                lhsT=wr_l,
                                rhs=hin[:, G * 512 : (G + 1) * 512],
                                start=True,
                                stop=False,
                            )
                            nc.tensor.matmul(
                                out=pr_[:],
                                lhsT=yr2[:],
                                rhs=npad_sb[0:2, G * 512 : (G + 1) * 512],
                                start=False,
                                stop=False,
                            )
                            # spill one-hots + scatter, per half
                            n_ch = 2 * s_ch  # 2 groups x ch
                            for h in range(2):
                                oh = ohp.tile([128, n_ch, SG], BF16, tag="oh")
                                d0 = (h * NSG + 2 * G) * s_ch
                                nc.vector.tensor_tensor(
                                    out=oh[:],
                                    in0=iota_sb[:]
                                    .rearrange("p (c f) -> p c f", c=1)
                                    .to_broadcast([128, n_ch, SG]),
                                    in1=drsp_sb[:, d0 : d0 + n_ch]
                                    .rearrange("p (c f) -> p c f", f=1)
                                    .to_broadcast([128, n_ch, SG]),
                                    op=OP.is_equal,
                                )
                                if not skip_gather:
                                    for i in range(n_ch):
                                        g = 2 * G + i // s_ch
                                        ch = i % s_ch
                                        pos = (g - o * 4) * s_ch + ch
                                        t = msp_lo_t if h == 0 else msp_hi
                                        nc.tensor.matmul(
                                            out=pr_[
                                                :,
                                                (g % 2) * SG : (g % 2 + 1) * SG,
                                            ],
                                            lhsT=t[:, pos, :],
                                            rhs=oh[:, i, :],
                                            start=False,
                                            stop=False,
                                        )
                            # agg transposes: 4 chunks of 128 nodes
                            for j in range(4):
                                nc.tensor.matmul(
                                    out=pr_[:, j * 128 : (j + 1) * 128],
                                    lhsT=agg[:, G * 4 + j, :],
                                    rhs=id_sb[:],
                                    start=False,
                                    stop=(j == 3),
                                )
                            nc.scalar.activation(
                                out=h_raw[:, G * 512 : (G + 1) * 512],
                                in_=pr_[:],
                                func=AF.Copy,
                            )

                    # E) BN statistics (sum, sumsq) + AllReduce
                    stats = sp.tile([128, 2], F32, tag="stats")
                    nc.vector.tensor_reduce(
                        out=stats[:, 0:1],
                        in_=h_raw[:],
                        axis=mybir.AxisListType.X,
                        op=OP.add,
                    )
                    sq = sqp.tile([128, 512], F32, tag="sq")
                    s2p = sp.tile([128, 16], F32, tag="s2p")
                    for t in range(16):
                        nc.vector.tensor_tensor(
                            out=sq[:],
                            in0=h_raw[:, t * 512 : (t + 1) * 512],
                            in1=h_raw[:, t * 512 : (t + 1) * 512],
                            op=OP.mult,
                        )
                        nc.vector.tensor_reduce(
                            out=s2p[:, t : t + 1],
                            in_=sq[:],
                            axis=mybir.AxisListType.X,
                            op=OP.add,
                        )
                    nc.vector.tensor_reduce(
                        out=stats[:, 1:2],
                        in_=s2p[:],
                        axis=mybir.AxisListType.X,
                        op=OP.add,
                    )
                    nc.sync.dma_start(out=bn_in[l], in_=stats[:])
                    nc.gpsimd.collective_compute(
                        "AllReduce",
                        OP.add,
                        replica_groups=rg,
                        ins=[bn_in[l].opt()],
                        outs=[bn_out[l].opt()],
                    )
                    gstats = sp.tile([128, 2], F32, tag="gstats")
                    nc.sync.dma_start(out=gstats[:], in_=bn_out[l])

                    # F) scale/shift: m = s1/N + cb; v = s2/N - (s1/N)^2
                    pr = sp.tile([128, 6], F32, tag="bnpar")
                    nc.vector.tensor_scalar_mul(pr[:, 0:1], gstats[:, 0:1], 1.0 / N)
                    nc.vector.tensor_scalar_mul(pr[:, 1:2], gstats[:, 1:2], 1.0 / N)
                    nc.vector.tensor_tensor(
                        out=pr[:, 2:3], in0=pr[:, 0:1], in1=pr[:, 0:1], op=OP.mult
                    )
                    nc.vector.tensor_tensor(
                        out=pr[:, 1:2], in0=pr[:, 1:2], in1=pr[:, 2:3],
                        op=OP.subtract,
                    )
                    nc.vector.tensor_scalar_add(pr[:, 1:2], pr[:, 1:2], EPS)
                    nc.scalar.sqrt(out=pr[:, 2:3], in_=pr[:, 1:2])
                    nc.vector.reciprocal(out=pr[:, 3:4], in_=pr[:, 2:3])
                    nc.vector.tensor_tensor(
                        out=pr[:, 3:4], in0=pr[:, 3:4],
                        in1=bng_sb[:, l : l + 1], op=OP.mult,
                    )
                    nc.vector.tensor_tensor(
                        out=pr[:, 0:1], in0=pr[:, 0:1],
                        in1=cb_sb[:, l : l + 1], op=OP.add,
                    )
                    nc.vector.tensor_tensor(
                        out=pr[:, 4:5], in0=pr[:, 0:1], in1=pr[:, 3:4], op=OP.mult
                    )
                    nc.vector.tensor_tensor(
                        out=pr[:, 5:6], in0=bnb_sb[:, l : l + 1],
                        in1=pr[:, 4:5], op=OP.subtract,
                    )

                    # G) h = relu(h_raw * scale + shift), bf16
                    for t in range(4):
                        nc.scalar.activation(
                            out=h_bf[:, t * 2048 : (t + 1) * 2048],
                            in_=h_raw[:, t * 2048 : (t + 1) * 2048],
                            func=AF.Relu,
                            bias=pr[:, 5:6],
                            scale=pr[:, 3:4],
                        )

            # --- heads (layers-scope pools are closed; SBUF freed)
            with (
                tc.tile_pool(name="hw", bufs=1) as hwp,
                tc.tile_pool(name="hsb", bufs=3) as hsb,
                tc.tile_pool(name="hps", bufs=2, space="PSUM") as hps,
                tc.tile_pool(name="hcst", bufs=1) as hc,
            ):
                # prefetch all node-head weights up front (SBUF is free now)
                PCHUNK = 16
                w1t, w2t = [], []
                for pc in range(NPG // PCHUNK):
                    w1 = hwp.tile([128, PCHUNK * 128], BF16, tag=f"w1_{pc}")
                    nc.sync.dma_start(
                        out=w1[:],
                        in_=nhw1[:, pc * PCHUNK * 128 : (pc + 1) * PCHUNK * 128],
                    )
                    w1t.append(w1)
                    w2 = hwp.tile([128, PCHUNK * 64], BF16, tag=f"w2_{pc}")
                    nc.sync.dma_start(
                        out=w2[:],
                        in_=nhw2[:, pc * PCHUNK * 64 : (pc + 1) * PCHUNK * 64],
                    )
                    w2t.append(w2)

                # graph pooling (contiguous 128-node graphs) + graph head
                pool = hc.tile([128, GPC], F32)
                nc.vector.tensor_reduce(
                    out=pool[:],
                    in_=h_bf[:].rearrange("p (g n) -> p g n", g=GPC),
                    axis=mybir.AxisListType.X,
                    op=OP.add,
                )
                gw = {}
                for name, apw, shape in (
                    ("gsw1", gsw1, [128, 128]),
                    ("gsw2", gsw2, [128, 128]),
                    ("ghw1", ghw1, [128, 128]),
                    ("ghw2", ghw2, [128, 64]),
                    ("ghw3", ghw3, [64, 2]),
                ):
                    t = hc.tile(shape, BF16, tag=name)
                    nc.sync.dma_start(out=t[:], in_=apw)
                    gw[name] = t
                gb = {}
                for name, apb, p in (
                    ("gsb1", gsb1, 128),
                    ("gsb2", gsb2, 128),
                    ("ghb1", ghb1, 128),
                    ("ghb2", ghb2, 64),
                    ("ghb3", ghb3, 2),
                ):
                    t = hc.tile([p, 1], F32, tag=name)
                    nc.sync.dma_start(out=t[:], in_=apb)
                    gb[name] = t

                g0 = hsb.tile([128, GPC], BF16, tag="g0")
                nc.scalar.activation(
                    out=g0[:], in_=pool[:], func=AF.Relu, scale=1.0 / NPG
                )
                gp1 = hps.tile([128, GPC], F32, space="PSUM", tag="gps")
                nc.tensor.matmul(
                    out=gp1[:], lhsT=gw["gsw1"][:], rhs=g0[:], start=True, stop=True
                )
                g1 = hsb.tile([128, GPC], BF16, tag="g1")
                nc.vector.tensor_scalar_add(g1[:], gp1[:], gb["gsb1"][:])
                gp2 = hps.tile([128, GPC], F32, space="PSUM", tag="gps")
                nc.tensor.matmul(
                    out=gp2[:], lhsT=gw["gsw2"][:], rhs=g1[:], start=True, stop=True
                )
                g2 = hsb.tile([128, GPC], BF16, tag="g2")
                nc.scalar.activation(
                    out=g2[:], in_=gp2[:], func=AF.Relu, bias=gb["gsb2"][:]
                )
                gp3 = hps.tile([128, GPC], F32, space="PSUM", tag="gps")
                nc.tensor.matmul(
                    out=gp3[:], lhsT=gw["ghw1"][:], rhs=g2[:], start=True, stop=True
                )
                g3 = hsb.tile([128, GPC], BF16, tag="g3")
                nc.scalar.activation(
                    out=g3[:], in_=gp3[:], func=AF.Relu, bias=gb["ghb1"][:]
                )
                gp4 = hps.tile([64, GPC], F32, space="PSUM", tag="gps")
                nc.tensor.matmul(
                    out=gp4[:], lhsT=gw["ghw2"][:], rhs=g3[:], start=True, stop=True
                )
                g4 = hsb.tile([64, GPC], BF16, tag="g4")
                nc.scalar.activation(
                    out=g4[:], in_=gp4[:], func=AF.Relu, bias=gb["ghb2"][:]
                )
                gp5 = hps.tile([2, GPC], F32, space="PSUM", tag="gps")
                nc.tensor.matmul(
                    out=gp5[:], lhsT=gw["ghw3"][:], rhs=g4[:], start=True, stop=True
                )
                gout = hsb.tile([2, GPC], F32, tag="gout")
                nc.vector.tensor_scalar_add(gout[:], gp5[:], gb["ghb3"][:])
                nc.sync.dma_start(out=outg, in_=gout[:])

                # node heads: 128 positions x (128->128->64->1), 64 graphs each
                nb1 = hc.tile([128, 128], F32, tag="nb1")
                nc.sync.dma_start(out=nb1[:], in_=nhb1)
                nb2 = hc.tile([64, 128], F32, tag="nb2")
                nc.sync.dma_start(out=nb2[:], in_=nhb2)
                nb3 = hc.tile([1, 128], F32, tag="nb3")
                nc.sync.dma_start(out=nb3[:], in_=nhb3)
                w3 = hc.tile([64, 128], BF16, tag="w3")
                nc.sync.dma_start(out=w3[:], in_=nhw3)
                # single-partition accumulator: engines can't write at a
                # nonzero partition offset, so row p lives at cols [p*GPC,...)
                out_n = hc.tile([1, NPG * GPC], F32, tag="out_n")

                for pc in range(NPG // PCHUNK):
                    w1, w2 = w1t[pc], w2t[pc]
                    for pi in range(PCHUNK):
                        p = pc * PCHUNK + pi
                        zp1 = hps.tile([128, GPC], F32, space="PSUM", tag="zp1")
                        nc.tensor.matmul(
                            out=zp1[:],
                            lhsT=w1[:, pi * 128 : (pi + 1) * 128],
                            rhs=h_bf[:, p :: NPG],
                            start=True,
                            stop=True,
                        )
                        z1 = hsb.tile([128, GPC], BF16, tag="z1")
                        nc.scalar.activation(
                            out=z1[:], in_=zp1[:], func=AF.Relu,
                            bias=nb1[:, p : p + 1],
                        )
                        zp2 = hps.tile([64, GPC], F32, space="PSUM", tag="zp2")
                        nc.tensor.matmul(
                            out=zp2[:],
                            lhsT=w2[:, pi * 64 : (pi + 1) * 64],
                            rhs=z1[:],
                            start=True,
                            stop=True,
                        )
                        z2 = hsb.tile([64, GPC], BF16, tag="z2")
                        nc.scalar.activation(
                            out=z2[:], in_=zp2[:], func=AF.Relu,
                            bias=nb2[:, p : p + 1],
                        )
                        zp3 = hps.tile([1, GPC], F32, space="PSUM", tag="zp3")
                        nc.tensor.matmul(
                            out=zp3[:],
                            lhsT=w3[:, p : p + 1],
                            rhs=z2[:],
                            start=True,
                            stop=True,
                        )
                        nc.vector.tensor_scalar_add(
                            out_n[:, p * GPC : (p + 1) * GPC],
                            zp3[:],
                            nb3[:, p : p + 1],
                        )
                nc.sync.dma_start(
                    out=outn.rearrange("(o p) g -> o (p g)", o=1), in_=out_n[:]
                )

    nc.compile()


# ------------------------------------------------------------------- driver

def _prep_inputs(inputs):
    f32 = lambda k: np.asarray(inputs[k], np.float32)
    bf16 = lambda a: np.ascontiguousarray(a).astype(NBF)

    edge_index = np.asarray(inputs["edge_index"], np.int64)
    s_ch, idx_slab, negpad, sp_idx, sp_drel = _build_edge_plan(edge_index)

    x = f32("x")
    iota = np.tile(np.arange(SG, dtype=np.float32), (128, 1))

    shared = {
        "iota": bf16(iota),
        "ident": bf16(np.eye(128, dtype=np.float32)),
        "wn0": bf16(f32("conv0_wn")),
        "wr0": bf16(f32("conv0_wr")),
        "wn12": bf16(f32("convs_wn")),
        "wr12": bf16(f32("convs_wr")),
        "cb": np.stack(
            [f32("conv0_b"), f32("convs_b")[0], f32("convs_b")[1]], axis=1
        ).copy(),
        "bng": np.stack(
            [f32("bn0_g"), f32("bns_g")[0], f32("bns_g")[1]], axis=1
        ).copy(),
        "bnb": np.stack(
            [f32("bn0_b"), f32("bns_b")[0], f32("bns_b")[1]], axis=1
        ).copy(),
        "gsw1": bf16(f32("gs_w1")),
        "gsw2": bf16(f32("gs_w2")),
        "ghw1": bf16(f32("gh_w1")),
        "ghw2": bf16(f32("gh_w2")),
        "ghw3": bf16(f32("gh_w3")),
        "gsb1": f32("gs_b1").reshape(128, 1).copy(),
        "gsb2": f32("gs_b2").reshape(128, 1).copy(),
        "ghb1": f32("gh_b1").reshape(128, 1).copy(),
        "ghb2": f32("gh_b2").reshape(64, 1).copy(),
        "ghb3": f32("gh_b3").reshape(2, 1).copy(),
        "nhw1": bf16(f32("nh_w1").transpose(1, 0, 2).reshape(128, 128 * 128)),
        "nhw2": bf16(f32("nh_w2").transpose(1, 0, 2).reshape(128, 128 * 64)),
        "nhw3": bf16(f32("nh_w3")[:, :, 0].T),
        "nhb1": f32("nh_b1").T.copy(),
        "nhb2": f32("nh_b2").T.copy(),
        "nhb3": f32("nh_b3").T.copy(),
    }

    in_maps = []
    for c in range(NC):
        idx_w = _wrap_calls(idx_slab[c].reshape(-1), OCT).copy()
        sp_w = _wrap_calls(sp_idx[c].reshape(-1), SCALL).copy()
        # spill dstrel, chunk-major columns: col = (h*NSG + g)*s_ch + ch
        dr = sp_drel[c].reshape(-1, 128).T
        in_maps.append(
            dict(
                shared,
                xin=bf16(x[c * NPC : (c + 1) * NPC].T),
                idxs=idx_w,
                spidx=sp_w,
                drsp=dr.astype(NBF).copy(),
                npad=negpad[c].astype(NBF).copy(),
            )
        )
    return s_ch, in_maps


def _numpy_fallback(inputs):
    """Reference math in numpy for unexpected input layouts."""
    f = lambda k: np.asarray(inputs[k], np.float32)
    x = f("x")
    src, dst = np.asarray(inputs["edge_index"], np.int64)
    batch = np.asarray(inputs["batch"], np.int64)

    def gconv(h, wr, wn, b):
        y = h @ wn
        agg = np.zeros_like(h @ wr)
        np.add.at(agg, dst, y[src])
        return h @ wr + agg + b

    def bn(h, g, bt):
        m = h.mean(0)
        v = h.var(0)
        return (h - m) / np.sqrt(v + EPS) * g + bt

    h = np.maximum(bn(gconv(x, f("conv0_wr"), f("conv0_wn"), f("conv0_b")),
                      f("bn0_g"), f("bn0_b")), 0)
    for i in range(2):
        h = np.maximum(
            bn(gconv(h, f("convs_wr")[i], f("convs_wn")[i], f("convs_b")[i]),
               f("bns_g")[i], f("bns_b")[i]), 0)
    counts = np.bincount(batch, minlength=B).astype(np.float32)
    xg = np.zeros((B, H), np.float32)
    np.add.at(xg, batch, h)
    xg /= counts[:, None]
    g = np.maximum(xg, 0)
    g = g @ f("gs_w1") + f("gs_b1")
    g = np.maximum(g @ f("gs_w2") + f("gs_b2"), 0)
    g = np.maximum(g @ f("gh_w1") + f("gh_b1"), 0)
    g = np.maximum(g @ f("gh_w2") + f("gh_b2"), 0)
    g = g @ f("gh_w3") + f("gh_b3")
    xn = h.reshape(B, NPG, H)
    z = np.maximum(np.einsum("bnf,nfh->bnh", xn, f("nh_w1")) + f("nh_b1"), 0)
    z = np.maximum(np.einsum("bnh,nhk->bnk", z, f("nh_w2")) + f("nh_b2"), 0)
    z = np.einsum("bnk,nko->bno", z, f("nh_w3")) + f("nh_b3")
    return np.concatenate([g, z[:, :, 0]], axis=1).astype(np.float32)


def _run(inputs, trace=False, trace_kwargs=None):
    batch = np.asarray(inputs["batch"], np.int64)
    if not (
        np.array_equal(batch, np.arange(N, dtype=np.int64) // NPG)
        and np.asarray(inputs["x"]).shape == (N, 32)
        and np.asarray(inputs["edge_index"]).shape == (2, E)
    ):
        return _numpy_fallback(inputs), None

    s_ch, in_maps = _prep_inputs(inputs)
    nc = bacc.Bacc(
        "TRN2",
        target_bir_lowering=False,
        debug=False,
        num_devices=NC,
        num_swdge_queues=NQ,
    )
    _build(nc, s_ch)
    r = run_bass_kernel_spmd(
        nc, in_maps, list(range(NC)), trace=trace, **(trace_kwargs or {})
    )
    out = np.zeros((B, 2 + NPG), np.float32)
    for c in range(NC):
        out[c * GPC : (c + 1) * GPC, 0:2] = r.results[c]["outg"].T
        out[c * GPC : (c + 1) * GPC, 2:] = r.results[c]["outn"].T
    return out, r


def kernel(**inputs):
    out, _ = _run(inputs)
    return out


# revision 25
# speedup vs baseline: 1.4325x; 1.0540x over previous
"""GNN message-passing kernel for 8 TRN2 NeuronCores (Bass/Tile, SPMD).

Takes the FULL inputs of nn_Base_40793599378196 and returns the FULL
[512, 130] output. Internally:

- Nodes/graphs sharded by graph: core c owns nodes [c*8192, (c+1)*8192).
  Weights replicated. Per layer y = h @ wn is computed locally (node-major
  bf16), AllGathered into a full [65536, 128] DRAM table.
- Aggregation avoids both the random-256B-HBM-read drain floor (~105ns per
  descriptor per SDMA engine) and the one-hot/scatter-matmul volume:
  * Each 32768-node half of the table is DMA'd contiguously into an SBUF
    table (8MB, token t = y row (t%128)*256 + t//128 at partition t%128,
    stripe t//128 -- so the idx for row r is (r%256)*128 + r//256).
  * Edges are organized per (dst node, src half) into K=4 fixed slots;
    slot-slab streams are gathered EDGE-MAJOR with a non-transpose
    SBUF-source dma_gather (constructed directly -- the bass helper only
    allows SBUF sources with transpose=True, but the firmware supports
    non-transpose too, avoiding the xbar so the 4 SWDGE queues can
    generate descriptors concurrently; SBUF reads make the drain cheap).
  * The K slabs accumulate into agg[node%128, node//128, feat] with plain
    DVE adds; per 512-node group a transpose-matmul (agg_chunk^T @ I)
    accumulates agg into the root-transform PSUM group. Pad slots gather
    token 0 and are cancelled by a rank-1 correction matmul.
  * Overflow edges (degree > K per half, ~2.5%) are gathered from DRAM
    y_full (edge-major chunks per 256-node spill group) and scattered via
    one-hot (iota256 is_equal) matmuls into the same PSUM groups.
- Gathers rotate over the 4 SWDGE queues (4 Q7 core pairs in parallel),
  single_packet=True, <=1024 idx per call.
- BatchNorm statistics are AllReduced; conv bias folds into the BN shift.
- Graph pooling = free-dim window reduction; graph head + 128 per-node
  head MLPs run on the 64 local graphs.

Compute dtype: bf16 operands with fp32 PSUM/statistics.
"""

import os

import numpy as np
import ml_dtypes

import concourse.bacc as bacc
import concourse.tile as tile
import concourse.mybir as mybir
from concourse.bass_utils import run_bass_kernel_spmd

F32 = mybir.dt.float32
BF16 = mybir.dt.bfloat16
I16 = mybir.dt.int16
AF = mybir.ActivationFunctionType
OP = mybir.AluOpType

NBF = ml_dtypes.bfloat16

N = 65536
E = 524288
H = 128
B = 512
NPG = 128
NC = 8
NPC = N // NC      # 8192 nodes per core
HALF = N // 2
GPC = B // NC      # 64 graphs per core
EPS = 1e-5
NQ = 4             # SWDGE queues

K = 3              # tree slots per (node, half)
SG = 256           # spill group width (nodes)
NSG = NPC // SG    # 32 spill groups
SCALL = 1024       # idx per gather call
OCT = 1024         # nodes per "octet" (one slab call per slab)
NOCT = NPC // OCT  # 8


# ----------------------------------------------------------------- host prep

def _build_edge_plan(edge_index):
    src = edge_index[0].astype(np.int64)
    dst = edge_index[1].astype(np.int64)
    core = dst // NPC
    nloc = dst % NPC
    hh = (src >= HALF).astype(np.int64)
    soff = src - hh * HALF

    key = (core * 2 + hh) * NPC + nloc
    order = np.argsort(key, kind="stable")
    soff_s = soff[order]

    cnt = np.bincount(key[order], minlength=NC * 2 * NPC)
    starts = np.zeros(NC * 2 * NPC + 1, np.int64)
    np.cumsum(cnt, out=starts[1:])
    cnt3 = cnt.reshape(NC, 2, NPC)

    tok = (soff_s % 256) * 128 + soff_s // 256  # SBUF table token ids

    # slab stream layout per (core, half): oct-major:
    #   pos = oct*K*OCT + k*OCT + (n % OCT)
    idx_slab = np.zeros((NC, 2, NOCT, K, OCT), np.int16)
    negpad = np.zeros((NC, 2, NPC), np.float32)

    over = np.maximum(cnt3 - K, 0)
    spl_cnt = over.reshape(NC, 2, NSG, SG).sum(axis=3)
    s_ch = int(np.ceil(spl_cnt.max() / 128))
    sp_idx = np.zeros((NC, 2, NSG, s_ch * 128), np.int16)
    sp_drel = np.full((NC, 2, NSG, s_ch * 128), float(SG), np.float32)
    sp_fill = np.zeros((NC, 2, NSG), np.int64)

    for c in range(NC):
        for h in range(2):
            base = (c * 2 + h) * NPC
            for n in range(NPC):
                s0, s1 = starts[base + n], starts[base + n + 1]
                d = s1 - s0
                kk = min(d, K)
                o, nw = n // OCT, n % OCT
                idx_slab[c, h, o, :kk, nw] = tok[s0 : s0 + kk]
                negpad[c, h, n] = -(K - kk)
                if d > K:
                    g = n // SG
                    f = sp_fill[c, h, g]
                    m = d - K
                    # lo spills gather DRAM y_full rows (so they can run in
                    # the hi phase); hi spills gather the SBUF table (tokens)
                    vals = soff_s[s0 + K : s1] if h == 0 else tok[s0 + K : s1]
                    sp_idx[c, h, g, f : f + m] = vals
                    sp_drel[c, h, g, f : f + m] = n % SG
                    sp_fill[c, h, g] = f + m

    return s_ch, idx_slab, negpad, sp_idx, sp_drel


def _wrap_idx16(idx_flat):
    """[S] -> [128, S/16]: index i at partition i%16, col i//16, replicated
    over the 8 groups of 16 partitions."""
    s = idx_flat.shape[0]
    assert s % 16 == 0
    a = idx_flat.reshape(s // 16, 16).T
    return np.tile(a, (8, 1))


def _wrap_calls(flat, call):
    segs = [
        _wrap_idx16(flat[o : o + call]) for o in range(0, flat.shape[0], call)
    ]
    return np.concatenate(segs, axis=1)


# -------------------------------------------------------------- device build

def _sbuf_gather_nt(nc, out_ap, in_ap, idxs_ap, num_idxs, queue_num):
    """Non-transpose SBUF-source dma_gather. The bass helper only allows
    SBUF sources with transpose=True; the firmware decode supports the
    non-transpose combination (and it avoids the xbar, so multiple SWDGE
    queues can run concurrently). Construct the instruction directly."""
    eng = nc.gpsimd
    return eng.add_instruction(
        mybir.InstDMAGatherAnt(
            name=nc.get_next_instruction_name(),
            ins=[
                eng.lower_ap(in_ap),
                eng.lower_ap(idxs_ap),
                eng.lower_val_access(eng.to_reg(num_idxs)),
            ],
            outs=[eng.lower_ap(out_ap)],
            transpose=False,
            num_idxs=num_idxs,
            elem_size=128,
            stride_bytes_256=0,
            gen_mode=0,
            single_packet=True,
            queue_num=queue_num,
            sbuf_tokens_per_rank=128,
            sbuf_free_dim_per_rank=256,
            sbuf_free_dim_pad_per_rank=0,
            sbuf_byte_offset=0,
        )
    )


def _build(nc, s_ch):
    skip_gather = bool(int(os.environ.get("GNN_SKIP_GATHER", "0")))
    skip_cc = bool(int(os.environ.get("GNN_SKIP_CC", "0")))
    sp_per_h = NSG * s_ch * 128          # spill slots per half
    slab_cols = 2 * K * NPC // 16        # slab idx cols
    sp_cols = 2 * sp_per_h // 16
    sp_cpo = 4 * s_ch                    # spill chunks per oct (4 groups)

    def din(name, shape, dt):
        return nc.dram_tensor(name, shape, dt, kind="ExternalInput").ap()

    xin = din("xin", [32, NPC], BF16)
    idxs = din("idxs", [128, slab_cols], I16)
    spidx = din("spidx", [128, sp_cols], I16)
    drsp = din("drsp", [128, 2 * NSG * s_ch], BF16)
    iota = din("iota", [128, SG], BF16)
    ident = din("ident", [128, 128], BF16)
    npad = din("npad", [2, NPC], BF16)
    wn0 = din("wn0", [32, 128], BF16)
    wr0 = din("wr0", [32, 128], BF16)
    wn12 = din("wn12", [2, 128, 128], BF16)
    wr12 = din("wr12", [2, 128, 128], BF16)
    cb = din("cb", [128, 3], F32)
    bng = din("bng", [128, 3], F32)
    bnb = din("bnb", [128, 3], F32)
    gsw1 = din("gsw1", [128, 128], BF16)
    gsw2 = din("gsw2", [128, 128], BF16)
    ghw1 = din("ghw1", [128, 128], BF16)
    ghw2 = din("ghw2", [128, 64], BF16)
    ghw3 = din("ghw3", [64, 2], BF16)
    gsb1 = din("gsb1", [128, 1], F32)
    gsb2 = din("gsb2", [128, 1], F32)
    ghb1 = din("ghb1", [128, 1], F32)
    ghb2 = din("ghb2", [64, 1], F32)
    ghb3 = din("ghb3", [2, 1], F32)
    nhw1 = din("nhw1", [128, 128 * 128], BF16)
    nhw2 = din("nhw2", [128, 128 * 64], BF16)
    nhw3 = din("nhw3", [64, 128], BF16)
    nhb1 = din("nhb1", [128, 128], F32)
    nhb2 = din("nhb2", [64, 128], F32)
    nhb3 = din("nhb3", [1, 128], F32)

    outg = nc.dram_tensor("outg", [2, GPC], F32, kind="ExternalOutput").ap()
    outn = nc.dram_tensor("outn", [128, GPC], F32, kind="ExternalOutput").ap()

    y_local = nc.dram_tensor("y_local", [NPC, 128], BF16).ap()
    y_full = nc.dram_tensor("y_full", [N, 128], BF16, addr_space="Shared").ap()
    bn_in = [nc.dram_tensor(f"bn_in{l}", [128, 2], F32).ap() for l in range(3)]
    bn_out = [
        nc.dram_tensor(f"bn_out{l}", [128, 2], F32, addr_space="Shared").ap()
        for l in range(3)
    ]

    rg = [list(range(NC))]

    with tile.TileContext(nc) as tc:
        with (
            tc.tile_pool(name="persist", bufs=1) as pp,
            tc.tile_pool(name="small", bufs=2) as sp,
        ):
            # --- persistent tiles / constants
            # x lives in h_bf's first 32 partitions: the last read of x
            # (layer-0 root matmuls) precedes the first write of h_bf
            # (layer-0 BN activation)
            h_bf = pp.tile([128, NPC], BF16)
            nc.sync.dma_start(out=h_bf[:32, :], in_=xin)
            h_raw = pp.tile([128, NPC], F32)
            idx_sb = pp.tile([128, slab_cols], I16)
            nc.sync.dma_start(out=idx_sb[:], in_=idxs)
            spidx_sb = pp.tile([128, sp_cols], I16)
            nc.sync.dma_start(out=spidx_sb[:], in_=spidx)
            drsp_sb = pp.tile([128, 2 * NSG * s_ch], BF16)
            nc.sync.dma_start(out=drsp_sb[:], in_=drsp)
            iota_sb = pp.tile([128, SG], BF16)
            nc.sync.dma_start(out=iota_sb[:], in_=iota)
            id_sb = pp.tile([128, 128], BF16)
            nc.sync.dma_start(out=id_sb[:], in_=ident)
            npad_sb = pp.tile([2, NPC], BF16)
            nc.sync.dma_start(out=npad_sb[:], in_=npad)

            wn_sb = pp.tile([128, 3, 128], BF16)
            wr_sb = pp.tile([128, 3, 128], BF16)
            nc.sync.dma_start(out=wn_sb[:32, 0, :], in_=wn0)
            nc.sync.dma_start(out=wr_sb[:32, 0, :], in_=wr0)
            for l in range(2):
                nc.sync.dma_start(out=wn_sb[:, l + 1, :], in_=wn12[l])
                nc.sync.dma_start(out=wr_sb[:, l + 1, :], in_=wr12[l])
            cb_sb = pp.tile([128, 3], F32)
            nc.sync.dma_start(out=cb_sb[:], in_=cb)
            bng_sb = pp.tile([128, 3], F32)
            nc.sync.dma_start(out=bng_sb[:], in_=bng)
            bnb_sb = pp.tile([128, 3], F32)
            nc.sync.dma_start(out=bnb_sb[:], in_=bnb)

            # --- 3 GraphConv + BN + ReLU layers
            with (
                tc.tile_pool(name="ystage", bufs=1) as yp,
                tc.tile_pool(name="tab", bufs=1) as tbp,
                tc.tile_pool(name="mg", bufs=5) as mgp,
                tc.tile_pool(name="msp", bufs=4) as msppool,
                tc.tile_pool(name="ohsp", bufs=2) as ohp,
                tc.tile_pool(name="yr", bufs=2) as yrp,
                tc.tile_pool(name="sqp", bufs=1) as sqp,
                tc.tile_pool(name="psA", bufs=2, space="PSUM") as psA,
                tc.tile_pool(name="psR", bufs=3, space="PSUM") as psR,
            ):
                for l in range(3):
                    KIN = 32 if l == 0 else 128
                    hin = h_bf
                    wn_l = wn_sb[:KIN, l, :]
                    wr_l = wr_sb[:KIN, l, :]

                    # A) y_local = (h^T @ wn) node-major bf16
                    # (shares the buffer with agg: disjoint lifetimes)
                    ystage = yp.tile([128, NPC // 128, 128], BF16, tag="stg")
                    for blk in range(NPC // 128):
                        yps = psA.tile([128, 128], F32, space="PSUM", tag="yps")
                        nc.tensor.matmul(
                            out=yps[:],
                            lhsT=hin[:KIN, blk * 128 : (blk + 1) * 128],
                            rhs=wn_l,
                            start=True,
                            stop=True,
                        )
                        nc.scalar.activation(
                            out=ystage[:, blk, :], in_=yps[:], func=AF.Copy
                        )
                    nc.sync.dma_start(
                        out=y_local.rearrange("(b p) f -> p b f", p=128),
                        in_=ystage[:],
                    )

                    # B) replicate the message table
                    if not skip_cc:
                        nc.gpsimd.collective_compute(
                            "AllGather",
                            OP.bypass,
                            replica_groups=rg,
                            ins=[y_local.opt()],
                            outs=[y_full.opt()],
                        )
                    # pad-correction rows y[0], y[HALF] (stacked, K=2)
                    yr2 = yrp.tile([2, 128], BF16, tag="yr2")
                    nc.sync.dma_start(out=yr2[0:1, :], in_=y_full[0:1, :])
                    nc.sync.dma_start(out=yr2[1:2, :], in_=y_full[HALF : HALF + 1, :])

                    # agg[node%128, node//128, feat], accumulated over slabs
                    # (same pool/tag as ystage: reuses its buffer)
                    agg = yp.tile([128, NPC // 128, 128], BF16, tag="stg")
                    s1p = sp.tile([128, 16], F32, tag="s1p")
                    nc.vector.memset(s1p[:], 0.0)
                    s2p = sp.tile([128, 16], F32, tag="s2p")
                    nc.vector.memset(s2p[:], 0.0)
                    qn = 0

                    def slab_call(h, o, k):
                        nonlocal qn
                        c0 = (h * K * NPC + o * K * OCT + k * OCT) // 16
                        mg = mgp.tile([128, OCT // 128, 128], BF16, tag="mg")
                        if not skip_gather:
                            _sbuf_gather_nt(
                                nc, mg[:], T[:],
                                idx_sb[:, c0 : c0 + OCT // 16],
                                OCT, qn % NQ,
                            )
                        qn += 1
                        ab = agg[:, o * (OCT // 128) : (o + 1) * (OCT // 128), :]
                        if h == 0 and k == 0:
                            nc.vector.tensor_copy(out=ab, in_=mg[:])
                        elif not skip_gather:
                            nc.vector.tensor_tensor(
                                out=ab, in0=ab, in1=mg[:], op=OP.add
                            )

                    def spill_call(h, o):
                        # sub-calls of <=8 chunks (1024-idx single_packet cap)
                        # h==0: DRAM rows (usable in the hi phase); h==1: SBUF
                        # table tokens (cheap drains)
                        nonlocal qn
                        t = msppool.tile([128, sp_cpo, 128], BF16, tag="msp")
                        if not skip_gather:
                            for a in range(0, sp_cpo, 8):
                                b = min(a + 8, sp_cpo)
                                c0 = (h * sp_per_h + (o * sp_cpo + a) * 128) // 16
                                if h == 0:
                                    nc.gpsimd.dma_gather(
                                        t[:, a:b, :],
                                        y_full[:HALF],
                                        spidx_sb[:, c0 : c0 + (b - a) * 8],
                                        (b - a) * 128,
                                        (b - a) * 128,
                                        128,
                                        single_packet=True,
                                        queue_num=qn % NQ,
                                    )
                                else:
                                    _sbuf_gather_nt(
                                        nc,
                                        t[:, a:b, :],
                                        T[:],
                                        spidx_sb[:, c0 : c0 + (b - a) * 8],
                                        (b - a) * 128,
                                        qn % NQ,
                                    )
                                qn += 1
                        return t

                    # C) lo phase: table + slab gathers
                    T = tbp.tile([128, HALF // 128, 128], BF16, tag="T")
                    if not skip_gather:
                        nc.sync.dma_start(
                            out=T[:],
                            in_=y_full[:HALF].rearrange("(p s) f -> p s f", p=128),
                        )
                    for o in range(NOCT):
                        for k in range(K):
                            slab_call(0, o, k)

                    # D) hi phase: table swap + slab gathers + spills + PSUM
                    T = tbp.tile([128, HALF // 128, 128], BF16, tag="T")
                    if not skip_gather:
                        nc.sync.dma_start(
                            out=T[:],
                            in_=y_full[HALF:].rearrange("(p s) f -> p s f", p=128),
                        )
                    for o in range(NOCT):
                        msp_lo_t = spill_call(0, o)
                        msp_hi = spill_call(1, o)
                        for k in range(K):
                            slab_call(1, o, k)
                        # PSUM groups for this oct (2 x 512 nodes)
                        for Gi in range(2):
                            G = o * 2 + Gi
                            pr_ = psR.tile(
                                [128, 512], F32, space="PSUM", tag="pr"
                            )
                            nc.tensor.matmul(
                                out=pr_[:],
                                lhsT=wr_l,
                                rhs=hin[:KIN, G * 512 : (G + 1) * 512],
                                start=True,
                                stop=False,
                            )
                            nc.tensor.matmul(
                                out=pr_[:],
                                lhsT=yr2[:],
                                rhs=npad_sb[0:2, G * 512 : (G + 1) * 512],
                                start=False,
                                stop=False,
                            )
                            # spill one-hots + scatter, per half
                            n_ch = 2 * s_ch  # 2 groups x ch
                            for h in range(2):
                                oh = ohp.tile([128, n_ch, SG], BF16, tag="oh")
                                d0 = (h * NSG + 2 * G) * s_ch
                                nc.vector.tensor_tensor(
                                    out=oh[:],
                                    in0=iota_sb[:]
                                    .rearrange("p (c f) -> p c f", c=1)
                                    .to_broadcast([128, n_ch, SG]),
                                    in1=drsp_sb[:, d0 : d0 + n_ch]
                                    .rearrange("p (c f) -> p c f", f=1)
                                    .to_broadcast([128, n_ch, SG]),
                                    op=OP.is_equal,
                                )
                                if not skip_gather:
                                    for i in range(n_ch):
                                        g = 2 * G + i // s_ch
                                        ch = i % s_ch
                                        pos = (g - o * 4) * s_ch + ch
                                        t = msp_lo_t if h == 0 else msp_hi
                                        nc.tensor.matmul(
                                            out=pr_[
                                                :,
                                                (g % 2) * SG : (g % 2 + 1) * SG,
                                            ],
                                            lhsT=t[:, pos, :],
                                            rhs=oh[:, i, :],
                                            start=False,
                                            stop=False,
                                        )
                            # agg transposes: 4 chunks of 128 nodes
                            for j in range(4):
                                nc.tensor.matmul(
                                    out=pr_[:, j * 128 : (j + 1) * 128],
                                    lhsT=agg[:, G * 4 + j, :],
                                    rhs=id_sb[:],
                                    start=False,
                                    stop=(j == 3),
                                )
                            nc.scalar.activation(
                                out=h_raw[:, G * 512 : (G + 1) * 512],
                                in_=pr_[:],
                                func=AF.Copy,
                                accum_out=s1p[:, G : G + 1],
                            )
                            sq = sqp.tile([128, 512], F32, tag="sq")
                            nc.scalar.activation(
                                out=sq[:],
                                in_=h_raw[:, G * 512 : (G + 1) * 512],
                                func=AF.Square,
                                accum_out=s2p[:, G : G + 1],
                            )

                    # E) BN statistics from the per-group scalar partials
                    stats = sp.tile([128, 2], F32, tag="stats")
                    nc.vector.tensor_reduce(
                        out=stats[:, 0:1],
                        in_=s1p[:],
                        axis=mybir.AxisListType.X,
                        op=OP.add,
                    )
                    nc.vector.tensor_reduce(
                        out=stats[:, 1:2],
                        in_=s2p[:],
                        axis=mybir.AxisListType.X,
                        op=OP.add,
                    )
                    nc.sync.dma_start(out=bn_in[l], in_=stats[:])
                    nc.gpsimd.collective_compute(
                        "AllReduce",
                        OP.add,
                        replica_groups=rg,
                        ins=[bn_in[l].opt()],
                        outs=[bn_out[l].opt()],
                    )
                    gstats = sp.tile([128, 2], F32, tag="gstats")
                    nc.sync.dma_start(out=gstats[:], in_=bn_out[l])

                    # F) scale/shift: m = s1/N + cb; v = s2/N - (s1/N)^2
                    pr = sp.tile([128, 6], F32, tag="bnpar")
                    nc.vector.tensor_scalar_mul(pr[:, 0:1], gstats[:, 0:1], 1.0 / N)
                    nc.vector.tensor_scalar_mul(pr[:, 1:2], gstats[:, 1:2], 1.0 / N)
                    nc.vector.tensor_tensor(
                        out=pr[:, 2:3], in0=pr[:, 0:1], in1=pr[:, 0:1], op=OP.mult
                    )
                    nc.vector.tensor_tensor(
                        out=pr[:, 1:2], in0=pr[:, 1:2], in1=pr[:, 2:3],
                        op=OP.subtract,
                    )
                    nc.vector.tensor_scalar_add(pr[:, 1:2], pr[:, 1:2], EPS)
                    nc.scalar.sqrt(out=pr[:, 2:3], in_=pr[:, 1:2])
                    nc.vector.reciprocal(out=pr[:, 3:4], in_=pr[:, 2:3])
                    nc.vector.tensor_tensor(
                        out=pr[:, 3:4], in0=pr[:, 3:4],
                        in1=bng_sb[:, l : l + 1], op=OP.mult,
                    )
                    nc.vector.tensor_tensor(
                        out=pr[:, 0:1], in0=pr[:, 0:1],
                        in1=cb_sb[:, l : l + 1], op=OP.add,
                    )
                    nc.vector.tensor_tensor(
                        out=pr[:, 4:5], in0=pr[:, 0:1], in1=pr[:, 3:4], op=OP.mult
                    )
                    nc.vector.tensor_tensor(
                        out=pr[:, 5:6], in0=bnb_sb[:, l : l + 1],
                        in1=pr[:, 4:5], op=OP.subtract,
                    )

                    # G) h = relu(h_raw * scale + shift), bf16
                    for t in range(4):
                        nc.scalar.activation(
                            out=h_bf[:, t * 2048 : (t + 1) * 2048],
                            in_=h_raw[:, t * 2048 : (t + 1) * 2048],
                            func=AF.Relu,
                            bias=pr[:, 5:6],
                            scale=pr[:, 3:4],
                        )

            # --- heads (layers-scope pools are closed; SBUF freed)
            with (
                tc.tile_pool(name="hw", bufs=1) as hwp,
                tc.tile_pool(name="hsb", bufs=3) as hsb,
                tc.tile_pool(name="hps", bufs=2, space="PSUM") as hps,
                tc.tile_pool(name="hcst", bufs=1) as hc,
            ):
                # prefetch all node-head weights up front (SBUF is free now)
                PCHUNK = 16
                w1t, w2t = [], []
                for pc in range(NPG // PCHUNK):
                    w1 = hwp.tile([128, PCHUNK * 128], BF16, tag=f"w1_{pc}")
                    nc.sync.dma_start(
                        out=w1[:],
                        in_=nhw1[:, pc * PCHUNK * 128 : (pc + 1) * PCHUNK * 128],
                    )
                    w1t.append(w1)
                    w2 = hwp.tile([128, PCHUNK * 64], BF16, tag=f"w2_{pc}")
                    nc.sync.dma_start(
                        out=w2[:],
                        in_=nhw2[:, pc * PCHUNK * 64 : (pc + 1) * PCHUNK * 64],
                    )
                    w2t.append(w2)

                # graph pooling (contiguous 128-node graphs) + graph head
                pool = hc.tile([128, GPC], F32)
                nc.vector.tensor_reduce(
                    out=pool[:],
                    in_=h_bf[:].rearrange("p (g n) -> p g n", g=GPC),
                    axis=mybir.AxisListType.X,
                    op=OP.add,
                )
                gw = {}
                for name, apw, shape in (
                    ("gsw1", gsw1, [128, 128]),
                    ("gsw2", gsw2, [128, 128]),
                    ("ghw1", ghw1, [128, 128]),
                    ("ghw2", ghw2, [128, 64]),
                    ("ghw3", ghw3, [64, 2]),
                ):
                    t = hc.tile(shape, BF16, tag=name)
                    nc.sync.dma_start(out=t[:], in_=apw)
                    gw[name] = t
                gb = {}
                for name, apb, p in (
                    ("gsb1", gsb1, 128),
                    ("gsb2", gsb2, 128),
                    ("ghb1", ghb1, 128),
                    ("ghb2", ghb2, 64),
                    ("ghb3", ghb3, 2),
                ):
                    t = hc.tile([p, 1], F32, tag=name)
                    nc.sync.dma_start(out=t[:], in_=apb)
                    gb[name] = t

                g0 = hsb.tile([128, GPC], BF16, tag="g0")
                nc.scalar.activation(
                    out=g0[:], in_=pool[:], func=AF.Relu, scale=1.0 / NPG
                )
                gp1 = hps.tile([128, GPC], F32, space="PSUM", tag="gps")
                nc.tensor.matmul(
                    out=gp1[:], lhsT=gw["gsw1"][:], rhs=g0[:], start=True, stop=True
                )
                g1 = hsb.tile([128, GPC], BF16, tag="g1")
                nc.vector.tensor_scalar_add(g1[:], gp1[:], gb["gsb1"][:])
                gp2 = hps.tile([128, GPC], F32, space="PSUM", tag="gps")
                nc.tensor.matmul(
                    out=gp2[:], lhsT=gw["gsw2"][:], rhs=g1[:], start=True, stop=True
                )
                g2 = hsb.tile([128, GPC], BF16, tag="g2")
                nc.scalar.activation(
                    out=g2[:], in_=gp2[:], func=AF.Relu, bias=gb["gsb2"][:]
                )
                gp3 = hps.tile([128, GPC], F32, space="PSUM", tag="gps")
                nc.tensor.matmul(
                    out=gp3[:], lhsT=gw["ghw1"][:], rhs=g2[:], start=True, stop=True
                )
                g3 = hsb.tile([128, GPC], BF16, tag="g3")
                nc.scalar.activation(
                    out=g3[:], in_=gp3[:], func=AF.Relu, bias=gb["ghb1"][:]
                )
                gp4 = hps.tile([64, GPC], F32, space="PSUM", tag="gps")
                nc.tensor.matmul(
                    out=gp4[:], lhsT=gw["ghw2"][:], rhs=g3[:], start=True, stop=True
                )
                g4 = hsb.tile([64, GPC], BF16, tag="g4")
                nc.scalar.activation(
                    out=g4[:], in_=gp4[:], func=AF.Relu, bias=gb["ghb2"][:]
                )
                gp5 = hps.tile([2, GPC], F32, space="PSUM", tag="gps")
                nc.tensor.matmul(
                    out=gp5[:], lhsT=gw["ghw3"][:], rhs=g4[:], start=True, stop=True
                )
                gout = hsb.tile([2, GPC], F32, tag="gout")
                nc.vector.tensor_scalar_add(gout[:], gp5[:], gb["ghb3"][:])
                nc.sync.dma_start(out=outg, in_=gout[:])

                # node heads: 128 positions x (128->128->64->1), 64 graphs each
                nb1 = hc.tile([128, 128], F32, tag="nb1")
                nc.sync.dma_start(out=nb1[:], in_=nhb1)
                nb2 = hc.tile([64, 128], F32, tag="nb2")
                nc.sync.dma_start(out=nb2[:], in_=nhb2)
                nb3 = hc.tile([1, 128], F32, tag="nb3")
                nc.sync.dma_start(out=nb3[:], in_=nhb3)
                w3 = hc.tile([64, 128], BF16, tag="w3")
                nc.sync.dma_start(out=w3[:], in_=nhw3)
                # single-partition accumulator: engines can't write at a
                # nonzero partition offset, so row p lives at cols [p*GPC,...)
                out_n = hc.tile([1, NPG * GPC], F32, tag="out_n")

                for pc in range(NPG // PCHUNK):
                    w1, w2 = w1t[pc], w2t[pc]
                    for pi in range(PCHUNK):
                        p = pc * PCHUNK + pi
                        zp1 = hps.tile([128, GPC], F32, space="PSUM", tag="zp1")
                        nc.tensor.matmul(
                            out=zp1[:],
                            lhsT=w1[:, pi * 128 : (pi + 1) * 128],
                            rhs=h_bf[:, p :: NPG],
                            start=True,
                            stop=True,
                        )
                        z1 = hsb.tile([128, GPC], BF16, tag="z1")
                        nc.scalar.activation(
                            out=z1[:], in_=zp1[:], func=AF.Relu,
                            bias=nb1[:, p : p + 1],
                        )
                        zp2 = hps.tile([64, GPC], F32, space="PSUM", tag="zp2")
                        nc.tensor.matmul(
                            out=zp2[:],
                            lhsT=w2[:, pi * 64 : (pi + 1) * 64],
                            rhs=z1[:],
                            start=True,
                            stop=True,
                        )
                        z2 = hsb.tile([64, GPC], BF16, tag="z2")
                        nc.scalar.activation(
                            out=z2[:], in_=zp2[:], func=AF.Relu,
                            bias=nb2[:, p : p + 1],
                        )
                        zp3 = hps.tile([1, GPC], F32, space="PSUM", tag="zp3")
                        nc.tensor.matmul(
                            out=zp3[:],
                            lhsT=w3[:, p : p + 1],
                            rhs=z2[:],
                            start=True,
                            stop=True,
                        )
                        nc.vector.tensor_scalar_add(
                            out_n[:, p * GPC : (p + 1) * GPC],
                            zp3[:],
                            nb3[:, p : p + 1],
                        )
                nc.sync.dma_start(
                    out=outn.rearrange("(o p) g -> o (p g)", o=1), in_=out_n[:]
                )

    nc.compile()


# ------------------------------------------------------------------- driver

def _prep_inputs(inputs):
    f32 = lambda k: np.asarray(inputs[k], np.float32)
    bf16 = lambda a: np.ascontiguousarray(a).astype(NBF)

    edge_index = np.asarray(inputs["edge_index"], np.int64)
    s_ch, idx_slab, negpad, sp_idx, sp_drel = _build_edge_plan(edge_index)

    x = f32("x")
    iota = np.tile(np.arange(SG, dtype=np.float32), (128, 1))

    shared = {
        "iota": bf16(iota),
        "ident": bf16(np.eye(128, dtype=np.float32)),
        "wn0": bf16(f32("conv0_wn")),
        "wr0": bf16(f32("conv0_wr")),
        "wn12": bf16(f32("convs_wn")),
        "wr12": bf16(f32("convs_wr")),
        "cb": np.stack(
            [f32("conv0_b"), f32("convs_b")[0], f32("convs_b")[1]], axis=1
        ).copy(),
        "bng": np.stack(
            [f32("bn0_g"), f32("bns_g")[0], f32("bns_g")[1]], axis=1
        ).copy(),
        "bnb": np.stack(
            [f32("bn0_b"), f32("bns_b")[0], f32("bns_b")[1]], axis=1
        ).copy(),
        "gsw1": bf16(f32("gs_w1")),
        "gsw2": bf16(f32("gs_w2")),
        "ghw1": bf16(f32("gh_w1")),
        "ghw2": bf16(f32("gh_w2")),
        "ghw3": bf16(f32("gh_w3")),
        "gsb1": f32("gs_b1").reshape(128, 1).copy(),
        "gsb2": f32("gs_b2").reshape(128, 1).copy(),
        "ghb1": f32("gh_b1").reshape(128, 1).copy(),
        "ghb2": f32("gh_b2").reshape(64, 1).copy(),
        "ghb3": f32("gh_b3").reshape(2, 1).copy(),
        "nhw1": bf16(f32("nh_w1").transpose(1, 0, 2).reshape(128, 128 * 128)),
        "nhw2": bf16(f32("nh_w2").transpose(1, 0, 2).reshape(128, 128 * 64)),
        "nhw3": bf16(f32("nh_w3")[:, :, 0].T),
        "nhb1": f32("nh_b1").T.copy(),
        "nhb2": f32("nh_b2").T.copy(),
        "nhb3": f32("nh_b3").T.copy(),
    }

    in_maps = []
    for c in range(NC):
        idx_w = _wrap_calls(idx_slab[c].reshape(-1), OCT).copy()
        sp_w = _wrap_calls(sp_idx[c].reshape(-1), SCALL).copy()
        # spill dstrel, chunk-major columns: col = (h*NSG + g)*s_ch + ch
        dr = sp_drel[c].reshape(-1, 128).T
        in_maps.append(
            dict(
                shared,
                xin=bf16(x[c * NPC : (c + 1) * NPC].T),
                idxs=idx_w,
                spidx=sp_w,
                drsp=dr.astype(NBF).copy(),
                npad=negpad[c].astype(NBF).copy(),
            )
        )
    return s_ch, in_maps


def _numpy_fallback(inputs):
    """Reference math in numpy for unexpected input layouts."""
    f = lambda k: np.asarray(inputs[k], np.float32)
    x = f("x")
    src, dst = np.asarray(inputs["edge_index"], np.int64)
    batch = np.asarray(inputs["batch"], np.int64)

    def gconv(h, wr, wn, b):
        y = h @ wn
        agg = np.zeros_like(h @ wr)
        np.add.at(agg, dst, y[src])
        return h @ wr + agg + b

    def bn(h, g, bt):
        m = h.mean(0)
        v = h.var(0)
        return (h - m) / np.sqrt(v + EPS) * g + bt

    h = np.maximum(bn(gconv(x, f("conv0_wr"), f("conv0_wn"), f("conv0_b")),
                      f("bn0_g"), f("bn0_b")), 0)
    for i in range(2):
        h = np.maximum(
            bn(gconv(h, f("convs_wr")[i], f("convs_wn")[i], f("convs_b")[i]),
               f("bns_g")[i], f("bns_b")[i]), 0)
    counts = np.bincount(batch, minlength=B).astype(np.float32)
    xg = np.zeros((B, H), np.float32)
    np.add.at(xg, batch, h)
    xg /= counts[:, None]
    g = np.maximum(xg, 0)
    g = g @ f("gs_w1") + f("gs_b1")
    g = np.maximum(g @ f("gs_w2") + f("gs_b2"), 0)
    g = np.maximum(g @ f("gh_w1") + f("gh_b1"), 0)
    g = np.maximum(g @ f("gh_w2") + f("gh_b2"), 0)
    g = g @ f("gh_w3") + f("gh_b3")
    xn = h.reshape(B, NPG, H)
    z = np.maximum(np.einsum("bnf,nfh->bnh", xn, f("nh_w1")) + f("nh_b1"), 0)
    z = np.maximum(np.einsum("bnh,nhk->bnk", z, f("nh_w2")) + f("nh_b2"), 0)
    z = np.einsum("bnk,nko->bno", z, f("nh_w3")) + f("nh_b3")
    return np.concatenate([g, z[:, :, 0]], axis=1).astype(np.float32)


def _run(inputs, trace=False, trace_kwargs=None):
    batch = np.asarray(inputs["batch"], np.int64)
    if not (
        np.array_equal(batch, np.arange(N, dtype=np.int64) // NPG)
        and np.asarray(inputs["x"]).shape == (N, 32)
        and np.asarray(inputs["edge_index"]).shape == (2, E)
    ):
        return _numpy_fallback(inputs), None

    s_ch, in_maps = _prep_inputs(inputs)
    nc = bacc.Bacc(
        "TRN2",
        target_bir_lowering=False,
        debug=False,
        num_devices=NC,
        num_swdge_queues=NQ,
    )
    _build(nc, s_ch)
    r = run_bass_kernel_spmd(
        nc, in_maps, list(range(NC)), trace=trace, **(trace_kwargs or {})
    )
    out = np.zeros((B, 2 + NPG), np.float32)
    for c in range(NC):
        out[c * GPC : (c + 1) * GPC, 0:2] = r.results[c]["outg"].T
        out[c * GPC : (c + 1) * GPC, 2:] = r.results[c]["outn"].T
    return out, r


def kernel(**inputs):
    out, _ = _run(inputs)
    return out
